# revision 3
# baseline (speedup 1.0000x reference)
"""Trainium2 Bass kernel for nn_GAT_86045374808682 (3-layer GAT + coordinate head).

Self-contained: takes FULL inputs, shards across 8 NeuronCores internally,
returns the FULL [8192, 2] float32 output.

Strategy:
- Nodes relabeled by in-degree desc; 64 blocks of 128 striped across 8 cores
  (block j -> core j%8), so every core sees the same per-stripe padded degree
  schedule K[t] (SPMD: one program, identical shapes on all cores).
- Per-layer node table T[v] = [h(128) | sa(8) | da(8) | pad(48)] f32 (768B rows),
  row-gathered per edge-slot with gpsimd.dma_gather (dst-lane on partition,
  slots along free dim, slot-major index lists built on host).
- Edge phase per stripe, chunked by 16 slots: gather -> scores (narrow per-head)
  -> ex=exp(leaky_relu) -> w = h_g*ex_rep (DVE) -> PE transpose-accumulate over
  slots into PSUM -> divide by den (PE-replicated reciprocal) at stripe end.
- LN/ReLU in feature-major via PE ones-matmuls; rstd = exp(-0.5 ln(var+eps))
  with one Newton polish; tanh/softplus composed from exp/ln (single ACT table).
- 5 launches, 4 programs: P1 (x@W1 fp32), P2 x2 (edge+node+pack), P3 (edge+MLP
  head -> angles/radius), P4 (trig finalize, replicated). Host concats slabs.
"""
import sys

import numpy as np

for _p in ("/opt/trn_rl_repo", "/root/.axon_site/_ro/trn_rl_repo"):
    if _p not in sys.path:
        sys.path.append(_p)

import concourse.bass as bass  # noqa: F401
import concourse.tile as tile
from concourse import bacc, library_config, mybir
from concourse.masks import make_identity

dt = mybir.dt
AF = mybir.ActivationFunctionType
OP = mybir.AluOpType

N = 8192
IN = 8193
INP = 8320  # 65 * 128
H = 8
HC = 128
P = 128
NCORES = 8
NSTRIPE = 8
KC = 16  # gather chunk (slots)
MASKVAL = -1e5
PI = float(np.pi)


# ----------------------------------------------------------------------------
# host-side graph prep
# ----------------------------------------------------------------------------

def host_prep(src, dst):
    s = np.concatenate([np.asarray(src).astype(np.int64), np.arange(N, dtype=np.int64)])
    d = np.concatenate([np.asarray(dst).astype(np.int64), np.arange(N, dtype=np.int64)])
    deg = np.bincount(d, minlength=N)
    order = np.argsort(-deg, kind="stable")  # new-id -> old-id
    old2new = np.empty(N, np.int64)
    old2new[order] = np.arange(N)
    s_new = old2new[s]
    d_new = old2new[d]
    deg_new = deg[order]

    K = [int(deg_new[1024 * t]) for t in range(NSTRIPE)]  # desc-sorted -> stripe max
    offs = np.cumsum([0] + K)

    eo = np.argsort(d_new, kind="stable")
    s_sorted = s_new[eo]
    starts = np.searchsorted(d_new[eo], np.arange(N))

    idxq = np.zeros((NCORES, 16, int(offs[-1]) * 8), np.int16)
    maskq = np.full((NCORES, P, int(offs[-1])), MASKVAL, np.float32)
    ar = np.arange(P)
    for c in range(NCORES):
        for t in range(NSTRIPE):
            Kt = K[t]
            vids = (t * NCORES + c) * P + ar
            e0 = starts[vids]
            degs = deg_new[vids]
            kk = np.arange(Kt)
            take = np.minimum(e0[:, None] + kk[None, :], len(s_sorted) - 1)
            mat = s_sorted[take]                      # [128, Kt]
            valid = kk[None, :] < degs[:, None]
            mat = np.where(valid, mat, 0)
            maskq[c, :, offs[t] : offs[t] + Kt] = np.where(valid, 0.0, MASKVAL)
            lin = mat.T.reshape(-1)                   # slot-major [Kt*128]
            o16 = int(offs[t]) * 8
            idxq[c, :, o16 : o16 + Kt * 8] = lin.reshape(-1, 16).T
    return dict(order=order, K=K, offs=offs, idxq=idxq.astype(np.int16), maskq=maskq)


def core_cols(c):
    return np.concatenate([np.arange((t * NCORES + c) * P, (t * NCORES + c) * P + P)
                           for t in range(NSTRIPE)])


def mboth(a_src, a_dst):
    M = np.zeros((P, 16), np.float32)
    for h in range(H):
        M[h * 16 : (h + 1) * 16, h] = a_src[h]
        M[h * 16 : (h + 1) * 16, 8 + h] = a_dst[h]
    return M


# ----------------------------------------------------------------------------
# shared bass building blocks
# ----------------------------------------------------------------------------

def _mk_consts(nc, consts):
    c = {"pool": consts}
    nc.gpsimd.load_library(library_config.mlp)
    c["ident"] = consts.tile([P, P], dt.float32, name="c_ident")
    make_identity(nc, c["ident"][:])
    c["ones_col"] = consts.tile([P, 1], dt.float32, name="c_ones_col")
    nc.gpsimd.memset(c["ones_col"][:], 1.0)
    c["ones_row"] = consts.tile([1, P], dt.float32, name="c_ones_row")
    nc.gpsimd.memset(c["ones_row"][:], 1.0)
    c["eps"] = consts.tile([1, 1], dt.float32, name="c_eps")
    nc.gpsimd.memset(c["eps"][:], 1e-5)
    return c


def _rstd(nc, sb, var_ap, out_ap, n, eps):
    """out = 1/sqrt(var + eps): exp(-0.5 ln(var+eps)) + one Newton polish."""
    if eps:
        vpe = sb.tile([1, 512], dt.float32, tag="rs_vpe")
        nc.vector.tensor_scalar_add(vpe[:, 0:n], var_ap, float(eps))
        var_ap = vpe[:, 0:n]
    lnv = sb.tile([1, 512], dt.float32, tag="rs_ln")
    nc.scalar.activation(out=lnv[:, 0:n], in_=var_ap, func=AF.Ln)
    y = sb.tile([1, 512], dt.float32, tag="rs_y")
    nc.scalar.activation(out=y[:, 0:n], in_=lnv[:, 0:n], func=AF.Exp, scale=-0.5)
    u = sb.tile([1, 512], dt.float32, tag="rs_u")
    nc.vector.tensor_tensor(out=u[:, 0:n], in0=y[:, 0:n], in1=y[:, 0:n], op=OP.mult)
    nc.vector.tensor_tensor(out=u[:, 0:n], in0=u[:, 0:n], in1=var_ap, op=OP.mult)
    nc.vector.tensor_scalar(out=u[:, 0:n], in0=u[:, 0:n], scalar1=-0.5, scalar2=1.5,
                            op0=OP.mult, op1=OP.add)
    nc.vector.tensor_tensor(out=out_ap, in0=y[:, 0:n], in1=u[:, 0:n], op=OP.mult)


def _ln_relu_fm(nc, sb, ps, c, x_sb, n, gamma_t, beta_t, out_sb, nfeat=P):
    """Feature-major LN + affine + ReLU: out = relu(gamma*(x-mu)*rstd + beta).
    x_sb [nfeat, n] SBUF; per-column stats; processed in 512-col chunks."""
    for j in range(0, n, 512):
        w = min(512, n - j)
        xs = x_sb[:, j : j + w]
        xsq = sb.tile([nfeat, 512], dt.float32, tag="ln_xsq")
        nc.scalar.activation(out=xsq[:, 0:w], in_=xs, func=AF.Square)
        s1_ps = ps.tile([1, 512], dt.float32, space="PSUM", tag="pp_a")
        nc.tensor.matmul(out=s1_ps[:, 0:w], lhsT=c["ones_col"][0:nfeat, :], rhs=xs,
                         start=True, stop=True)
        s2_ps = ps.tile([1, 512], dt.float32, space="PSUM", tag="pp_b")
        nc.tensor.matmul(out=s2_ps[:, 0:w], lhsT=c["ones_col"][0:nfeat, :],
                         rhs=xsq[:, 0:w], start=True, stop=True)
        mu = sb.tile([1, 512], dt.float32, tag="ln_mu")
        nc.vector.tensor_scalar_mul(mu[:, 0:w], s1_ps[:, 0:w], 1.0 / nfeat)
        musq = sb.tile([1, 512], dt.float32, tag="ln_musq")
        nc.scalar.activation(out=musq[:, 0:w], in_=mu[:, 0:w], func=AF.Square)
        var = sb.tile([1, 512], dt.float32, tag="ln_var")
        nc.vector.scalar_tensor_tensor(out=var[:, 0:w], in0=s2_ps[:, 0:w],
                                       scalar=1.0 / nfeat, in1=musq[:, 0:w],
                                       op0=OP.mult, op1=OP.subtract)
        rs = sb.tile([1, 512], dt.float32, tag="ln_rs")
        _rstd(nc, sb, var[:, 0:w], rs[:, 0:w], w, 1e-5)
        rep_mu = ps.tile([nfeat, 512], dt.float32, space="PSUM", tag="pp_a")
        nc.tensor.matmul(out=rep_mu[:, 0:w], lhsT=c["ones_row"][:, 0:nfeat],
                         rhs=mu[:, 0:w], start=True, stop=True)
        rep_rs = ps.tile([nfeat, 512], dt.float32, space="PSUM", tag="pp_b")
        nc.tensor.matmul(out=rep_rs[:, 0:w], lhsT=c["ones_row"][:, 0:nfeat],
                         rhs=rs[:, 0:w], start=True, stop=True)
        xh = sb.tile([nfeat, 512], dt.float32, tag="ln_xh")
        nc.vector.tensor_tensor(out=xh[:, 0:w], in0=xs, in1=rep_mu[:, 0:w], op=OP.subtract)
        nc.vector.tensor_tensor(out=xh[:, 0:w], in0=xh[:, 0:w], in1=rep_rs[:, 0:w],
                                op=OP.mult)
        nc.scalar.activation(out=out_sb[:, j : j + w], in_=xh[:, 0:w], func=AF.Relu,
                             scale=gamma_t[:], bias=beta_t[:])


def _edge_stripe(nc, c, sb, gpool, wpool, ps, psagg, Tfull, idx_t, mask_t, da_stripe,
                 K_t, off_t, agg_sb, rep16_t):
    """One stripe: gather + segment softmax + weighted sum for 128 dst lanes.
    Writes normalized aggregation (feature-major [128 f, 128 dst]) to agg_sb."""
    nchunk = (K_t + KC - 1) // KC
    agg = psagg.tile([P, P], dt.float32, space="PSUM", tag="agg")
    den = sb.tile([P, 8], dt.float32, tag="den")
    for ci in range(nchunk):
        k0 = ci * KC
        kc = min(KC, K_t - k0)
        g = gpool.tile([P, KC, 192], dt.float32, tag="gather")
        nc.gpsimd.dma_gather(
            out_ap=g[:, 0:kc, :],
            in_ap=Tfull[:],
            idxs_ap=idx_t[:, (off_t + k0) * 8 : (off_t + k0 + kc) * 8],
            num_idxs=kc * P,
            num_idxs_reg=kc * P,
            elem_size=192,
            single_packet=False,
        )
        z = sb.tile([P, KC, 8], dt.float32, tag="z")
        nc.vector.tensor_tensor(out=z[:, 0:kc, :], in0=g[:, 0:kc, 128:136],
                                in1=da_stripe.unsqueeze(1).to_broadcast([P, kc, 8]),
                                op=OP.add)
        nc.vector.tensor_tensor(
            out=z[:, 0:kc, :], in0=z[:, 0:kc, :],
            in1=mask_t[:, off_t + k0 : off_t + k0 + kc].unsqueeze(2).to_broadcast([P, kc, 8]),
            op=OP.add)
        zl = sb.tile([P, KC, 8], dt.float32, tag="zl")
        nc.vector.tensor_scalar_mul(zl[:, 0:kc, :], z[:, 0:kc, :], 0.2)
        nc.vector.tensor_tensor(out=zl[:, 0:kc, :], in0=zl[:, 0:kc, :], in1=z[:, 0:kc, :],
                                op=OP.max)
        ex = sb.tile([P, KC, 8], dt.float32, tag="ex")
        nc.scalar.activation(out=ex[:, 0:kc, :], in_=zl[:, 0:kc, :], func=AF.Exp)
        dc = sb.tile([P, 8], dt.float32, tag="dc")
        nc.vector.tensor_reduce(out=dc[:], in_=ex[:, 0:kc, :].transpose([0, 2, 1]),
                                axis=mybir.AxisListType.X, op=OP.add)
        if ci == 0:
            nc.vector.tensor_copy(out=den[:], in_=dc[:])
        else:
            nc.vector.tensor_tensor(out=den[:], in0=den[:], in1=dc[:], op=OP.add)
        w = wpool.tile([P, KC, P], dt.float32, tag="w")
        nc.vector.tensor_tensor(
            out=w[:, 0:kc, :].rearrange("p k (h e) -> p k h e", h=8),
            in0=g[:, 0:kc, 0:128].rearrange("p k (h e) -> p k h e", h=8),
            in1=ex[:, 0:kc, :].unsqueeze(3).to_broadcast([P, kc, 8, 16]),
            op=OP.mult)
        for k in range(kc):
            nc.tensor.matmul(out=agg[:], lhsT=w[:, k, :], rhs=c["ident"][:],
                             is_transpose=True, start=(ci == 0 and k == 0),
                             stop=(ci == nchunk - 1 and k == kc - 1))
    dent = ps.tile([8, P], dt.float32, space="PSUM", tag="pp_a")
    nc.tensor.matmul(out=dent[0:8, :], lhsT=den[:], rhs=c["ident"][:],
                     is_transpose=True, start=True, stop=True)
    rden = sb.tile([8, P], dt.float32, tag="rden")
    nc.vector.reciprocal(out=rden[:], in_=dent[0:8, :])
    rdrep = ps.tile([P, P], dt.float32, space="PSUM", tag="pp_b")
    nc.tensor.matmul(out=rdrep[:], lhsT=rep16_t[:], rhs=rden[:], start=True, stop=True)
    rdrep_sb = sb.tile([P, P], dt.float32, tag="rdrep_sb")
    nc.vector.tensor_copy(out=rdrep_sb[:], in_=rdrep[:])
    nc.vector.tensor_tensor(out=agg_sb, in0=agg[:], in1=rdrep_sb[:], op=OP.mult)


# ----------------------------------------------------------------------------
# program builders
# ----------------------------------------------------------------------------

def build_p1():
    nc = bacc.Bacc(None, target_bir_lowering=False)
    xT = nc.declare_dram_parameter("xT", [INP, 1024], dt.float32, isOutput=False)
    W1 = nc.declare_dram_parameter("W1", [INP, HC], dt.float32, isOutput=False)
    Mb = nc.declare_dram_parameter("Mb", [P, 16], dt.float32, isOutput=False)
    Tout = nc.declare_dram_parameter("Tout", [1024, 192], dt.float32, isOutput=True)

    with tile.TileContext(nc) as tc:
        with (
            tc.tile_pool(name="consts", bufs=1) as consts,
            tc.tile_pool(name="wpool", bufs=3) as wp,
            tc.tile_pool(name="xpool", bufs=3) as xp,
            tc.tile_pool(name="sb", bufs=2) as sb,
            tc.tile_pool(name="psh", bufs=2, space="PSUM") as psh,
            tc.tile_pool(name="ps", bufs=2, space="PSUM") as ps,
        ):
            ident = consts.tile([P, P], dt.float32)
            make_identity(nc, ident[:])
            mb_t = consts.tile([P, 16], dt.float32)
            nc.sync.dma_start(out=mb_t[:], in_=Mb[:])
            for nb in range(2):
                hps = psh.tile([P, 512], dt.float32, space="PSUM", tag="hps")
                for kcb in range(65):
                    wt = wp.tile([P, P], dt.float32, tag="wt")
                    nc.sync.dma_start(out=wt[:], in_=W1[kcb * P : (kcb + 1) * P, :])
                    xt = xp.tile([P, 512], dt.float32, tag="xt")
                    nc.sync.dma_start(out=xt[:], in_=xT[kcb * P : (kcb + 1) * P,
                                                        nb * 512 : (nb + 1) * 512])
                    nc.tensor.matmul(out=hps[:], lhsT=wt[:], rhs=xt[:],
                                     start=(kcb == 0), stop=(kcb == 64))
                h_sb = sb.tile([P, 512], dt.float32, tag="h_sb")
                nc.vector.tensor_copy(out=h_sb[:], in_=hps[:])
                sada_ps = ps.tile([16, 512], dt.float32, space="PSUM", tag="pp_a")
                nc.tensor.matmul(out=sada_ps[0:16, :], lhsT=mb_t[:], rhs=h_sb[:],
                                 start=True, stop=True)
                sada_sb = sb.tile([16, 512], dt.float32, tag="sada_sb")
                nc.vector.tensor_copy(out=sada_sb[:], in_=sada_ps[0:16, :])
                for b in range(4):
                    blk = nb * 4 + b
                    ht_ps = ps.tile([P, P], dt.float32, space="PSUM", tag="pp_b")
                    nc.tensor.matmul(out=ht_ps[:], lhsT=h_sb[:, b * P : (b + 1) * P],
                                     rhs=ident[:], is_transpose=True, start=True, stop=True)
                    st_ps = ps.tile([P, 16], dt.float32, space="PSUM", tag="pp_c")
                    nc.tensor.matmul(out=st_ps[:], lhsT=sada_sb[:, b * P : (b + 1) * P],
                                     rhs=ident[0:16, 0:16], is_transpose=True,
                                     start=True, stop=True)
                    pk = sb.tile([P, 192], dt.float32, tag="pk")
                    nc.vector.tensor_copy(out=pk[:, 0:128], in_=ht_ps[:])
                    nc.vector.tensor_copy(out=pk[:, 128:144], in_=st_ps[:])
                    nc.vector.memset(pk[:, 144:192], 0.0)
                    nc.sync.dma_start(out=Tout[blk * P : (blk + 1) * P, :], in_=pk[:])
    nc.finalize()
    return nc


def build_p23(K, with_next, with_head):
    """P2 (with_next): edge agg + LN/ReLU/residual + W@ + sada + pack.
    P3 (with_head): edge agg + LN/ReLU/residual + row-norm + MLP head."""
    SK = int(sum(K))
    offs = np.cumsum([0] + list(K))
    nc = bacc.Bacc(None, target_bir_lowering=False)
    Tfull = nc.declare_dram_parameter("Tfull", [N, 192], dt.float32, isOutput=False)
    Town = nc.declare_dram_parameter("Town", [1024, 192], dt.float32, isOutput=False)
    xprev = nc.declare_dram_parameter("xprev", [P, 1024], dt.float32, isOutput=False)
    idxq = nc.declare_dram_parameter("idxq", [16, SK * 8], dt.int16, isOutput=False)
    maskq = nc.declare_dram_parameter("maskq", [P, SK], dt.float32, isOutput=False)
    bprev = nc.declare_dram_parameter("bprev", [P, 1], dt.float32, isOutput=False)
    gam = nc.declare_dram_parameter("gam", [P, 1], dt.float32, isOutput=False)
    bet = nc.declare_dram_parameter("bet", [P, 1], dt.float32, isOutput=False)
    rep16q = nc.declare_dram_parameter("rep16q", [8, P], dt.float32, isOutput=False)
    if with_next:
        Wn = nc.declare_dram_parameter("Wn", [P, P], dt.float32, isOutput=False)
        Mb = nc.declare_dram_parameter("Mb", [P, 16], dt.float32, isOutput=False)
        Tout = nc.declare_dram_parameter("Tout", [1024, 192], dt.float32, isOutput=True)
        xnout = nc.declare_dram_parameter("xnout", [P, 1024], dt.float32, isOutput=True)
    if with_head:
        aW1 = nc.declare_dram_parameter("aW1", [P, P], dt.float32, isOutput=False)
        ab1 = nc.declare_dram_parameter("ab1", [P, 1], dt.float32, isOutput=False)
        agm = nc.declare_dram_parameter("agm", [P, 1], dt.float32, isOutput=False)
        abe = nc.declare_dram_parameter("abe", [P, 1], dt.float32, isOutput=False)
        aW2 = nc.declare_dram_parameter("aW2", [P, 1], dt.float32, isOutput=False)
        ab2 = nc.declare_dram_parameter("ab2", [1, 1], dt.float32, isOutput=False)
        rW1 = nc.declare_dram_parameter("rW1", [P, 64], dt.float32, isOutput=False)
        rb1 = nc.declare_dram_parameter("rb1", [64, 1], dt.float32, isOutput=False)
        rgm = nc.declare_dram_parameter("rgm", [64, 1], dt.float32, isOutput=False)
        rbe = nc.declare_dram_parameter("rbe", [64, 1], dt.float32, isOutput=False)
        rW2 = nc.declare_dram_parameter("rW2", [64, 1], dt.float32, isOutput=False)
        rb2 = nc.declare_dram_parameter("rb2", [1, 1], dt.float32, isOutput=False)
        ang = nc.declare_dram_parameter("ang", [1, 1024], dt.float32, isOutput=True)
        rad = nc.declare_dram_parameter("rad", [1, 1024], dt.float32, isOutput=True)

    with tile.TileContext(nc) as tc:
        with (
            tc.tile_pool(name="consts", bufs=1) as consts,
            tc.tile_pool(name="gpool", bufs=3) as gpool,
            tc.tile_pool(name="wpool", bufs=2) as wpool,
            tc.tile_pool(name="sb", bufs=1) as sb,
            tc.tile_pool(name="ps", bufs=2, space="PSUM") as ps,
            tc.tile_pool(name="psagg", bufs=2, space="PSUM") as psagg,
        ):
            c = _mk_consts(nc, consts)
            rep16_t = consts.tile([8, P], dt.float32)
            nc.sync.dma_start(out=rep16_t[:], in_=rep16q[:])

            idx_t = sb.tile([P, SK * 8], dt.int16, tag="idx")
            for a in range(8):
                nc.sync.dma_start(out=idx_t[16 * a : 16 * (a + 1), :], in_=idxq[:])
            mask_t = sb.tile([P, SK], dt.float32, tag="mask")
            nc.sync.dma_start(out=mask_t[:], in_=maskq[:])
            da_t = sb.tile([P, NSTRIPE, 8], dt.float32, tag="da")
            nc.sync.dma_start(
                out=da_t[:],
                in_=Town[:].rearrange("(t p) r -> p t r", p=P)[:, :, 136:144])
            xprev_t = sb.tile([P, 1024], dt.float32, tag="xprev")
            nc.sync.dma_start(out=xprev_t[:], in_=xprev[:])
            bias_t = sb.tile([P, 1], dt.float32, tag="bias")
            nc.sync.dma_start(out=bias_t[:], in_=bprev[:])
            gam_t = sb.tile([P, 1], dt.float32, tag="gam")
            nc.sync.dma_start(out=gam_t[:], in_=gam[:])
            bet_t = sb.tile([P, 1], dt.float32, tag="bet")
            nc.sync.dma_start(out=bet_t[:], in_=bet[:])
            if with_next:
                wn_t = sb.tile([P, P], dt.float32, tag="wn")
                nc.sync.dma_start(out=wn_t[:], in_=Wn[:])
                mb_t = sb.tile([P, 16], dt.float32, tag="mb")
                nc.sync.dma_start(out=mb_t[:], in_=Mb[:])

            xnext = sb.tile([P, 1024], dt.float32, tag="xnext")

            for t in range(NSTRIPE):
                agg_sb = sb.tile([P, P], dt.float32, tag="agg_sb")
                _edge_stripe(nc, c, sb, gpool, wpool, ps, psagg, Tfull, idx_t, mask_t,
                             da_t[:, t, :], K[t], int(offs[t]), agg_sb[:], rep16_t)
                xb = sb.tile([P, P], dt.float32, tag="xb")
                nc.scalar.activation(out=xb[:], in_=agg_sb[:], func=AF.Identity,
                                     bias=bias_t[:], scale=1.0)
                xo = sb.tile([P, P], dt.float32, tag="xo")
                _ln_relu_fm(nc, sb, ps, c, xb[:], P, gam_t, bet_t, xo[:])
                nc.vector.tensor_tensor(out=xnext[:, t * P : (t + 1) * P], in0=xo[:],
                                        in1=xprev_t[:, t * P : (t + 1) * P], op=OP.add)

                if with_next:
                    hn_ps = ps.tile([P, P], dt.float32, space="PSUM", tag="pp_a")
                    nc.tensor.matmul(out=hn_ps[:], lhsT=wn_t[:],
                                     rhs=xnext[:, t * P : (t + 1) * P],
                                     start=True, stop=True)
                    hn_sb = sb.tile([P, P], dt.float32, tag="hn_sb")
                    nc.vector.tensor_copy(out=hn_sb[:], in_=hn_ps[:])
                    sada_ps = ps.tile([16, P], dt.float32, space="PSUM", tag="pp_b")
                    nc.tensor.matmul(out=sada_ps[0:16, :], lhsT=mb_t[:], rhs=hn_sb[:],
                                     start=True, stop=True)
                    sada_sb = sb.tile([16, P], dt.float32, tag="sada_sb")
                    nc.vector.tensor_copy(out=sada_sb[:], in_=sada_ps[0:16, :])
                    ht_ps = ps.tile([P, P], dt.float32, space="PSUM", tag="pp_a")
                    nc.tensor.matmul(out=ht_ps[:], lhsT=hn_sb[:], rhs=c["ident"][:],
                                     is_transpose=True, start=True, stop=True)
                    st_ps = ps.tile([P, 16], dt.float32, space="PSUM", tag="pp_b")
                    nc.tensor.matmul(out=st_ps[:], lhsT=sada_sb[:],
                                     rhs=c["ident"][0:16, 0:16], is_transpose=True,
                                     start=True, stop=True)
                    pk = sb.tile([P, 192], dt.float32, tag="pk")
                    nc.vector.tensor_copy(out=pk[:, 0:128], in_=ht_ps[:])
                    nc.vector.tensor_copy(out=pk[:, 128:144], in_=st_ps[:])
                    nc.vector.memset(pk[:, 144:192], 0.0)
                    nc.sync.dma_start(out=Tout[t * P : (t + 1) * P, :], in_=pk[:])

            if with_next:
                nc.sync.dma_start(out=xnout[:], in_=xnext[:])

            if with_head:
                n = 1024
                xsq = sb.tile([P, n], dt.float32, tag="hd_xsq")
                nc.scalar.activation(out=xsq[:], in_=xnext[:], func=AF.Square)
                h3n = sb.tile([P, n], dt.float32, tag="hd_h3n")
                for j in range(0, n, 512):
                    ss_ps = ps.tile([1, 512], dt.float32, space="PSUM", tag="pp_a")
                    nc.tensor.matmul(out=ss_ps[0:1, :], lhsT=c["ones_col"][:],
                                     rhs=xsq[:, j : j + 512], start=True, stop=True)
                    ss = sb.tile([1, 512], dt.float32, tag="hd_ss")
                    nc.vector.tensor_scalar_max(ss[:], ss_ps[0:1, :], 1e-24)
                    rn = sb.tile([1, 512], dt.float32, tag="hd_rn")
                    _rstd(nc, sb, ss[:], rn[:], 512, 0)
                    rn_rep = ps.tile([P, 512], dt.float32, space="PSUM", tag="pp_b")
                    nc.tensor.matmul(out=rn_rep[:], lhsT=c["ones_row"][:], rhs=rn[:],
                                     start=True, stop=True)
                    nc.vector.tensor_tensor(out=h3n[:, j : j + 512], in0=xnext[:, j : j + 512],
                                            in1=rn_rep[:], op=OP.mult)

                def mm_bias_act(lhsT_t, rhs_sb, m, bias_ap, out_sb):
                    for j in range(0, n, 512):
                        mm_ps = ps.tile([P, 512], dt.float32, space="PSUM", tag="pp_a")
                        nc.tensor.matmul(out=mm_ps[0:m, :], lhsT=lhsT_t,
                                         rhs=rhs_sb[:, j : j + 512], start=True, stop=True)
                        nc.scalar.activation(out=out_sb[:, j : j + 512], in_=mm_ps[0:m, :],
                                             func=AF.Identity, bias=bias_ap, scale=1.0)

                aW1_t = sb.tile([P, P], dt.float32, tag="hd_aW1")
                nc.sync.dma_start(out=aW1_t[:], in_=aW1[:])
                ab1_t = sb.tile([P, 1], dt.float32, tag="hd_ab1")
                nc.sync.dma_start(out=ab1_t[:], in_=ab1[:])
                agm_t = sb.tile([P, 1], dt.float32, tag="hd_agm")
                nc.sync.dma_start(out=agm_t[:], in_=agm[:])
                abe_t = sb.tile([P, 1], dt.float32, tag="hd_abe")
                nc.sync.dma_start(out=abe_t[:], in_=abe[:])
                a_pre = sb.tile([P, n], dt.float32, tag="hd_apre")
                mm_bias_act(aW1_t[:], h3n, P, ab1_t[:], a_pre)
                a_hid = sb.tile([P, n], dt.float32, tag="hd_ahid")
                _ln_relu_fm(nc, sb, ps, c, a_pre[:], n, agm_t, abe_t, a_hid[:])

                aW2_t = sb.tile([P, 1], dt.float32, tag="hd_aW2")
                nc.sync.dma_start(out=aW2_t[:], in_=aW2[:])
                ab2_t = sb.tile([1, 1], dt.float32, tag="hd_ab2")
                nc.sync.dma_start(out=ab2_t[:], in_=ab2[:])
                av = sb.tile([1, n], dt.float32, tag="hd_av")
                mm_bias_act(aW2_t[:], a_hid, 1, ab2_t[:], av)
                # angles = pi*tanh(av) = pi - 2pi/(exp(2av)+1)
                e2 = sb.tile([1, n], dt.float32, tag="hd_e2")
                nc.scalar.activation(out=e2[:], in_=av[:], func=AF.Exp, scale=2.0)
                nc.vector.tensor_scalar_add(e2[:], e2[:], 1.0)
                rr = sb.tile([1, n], dt.float32, tag="hd_rr")
                nc.vector.reciprocal(out=rr[:], in_=e2[:])
                angv = sb.tile([1, n], dt.float32, tag="hd_angv")
                nc.vector.tensor_scalar(out=angv[:], in0=rr[:], scalar1=-2.0 * PI,
                                        scalar2=PI, op0=OP.mult, op1=OP.add)
                nc.sync.dma_start(out=ang[:], in_=angv[:])

                rW1_t = sb.tile([P, 64], dt.float32, tag="hd_rW1")
                nc.sync.dma_start(out=rW1_t[:], in_=rW1[:])
                rb1_t = sb.tile([64, 1], dt.float32, tag="hd_rb1")
                nc.sync.dma_start(out=rb1_t[:], in_=rb1[:])
                rgm_t = sb.tile([64, 1], dt.float32, tag="hd_rgm")
                nc.sync.dma_start(out=rgm_t[:], in_=rgm[:])
                rbe_t = sb.tile([64, 1], dt.float32, tag="hd_rbe")
                nc.sync.dma_start(out=rbe_t[:], in_=rbe[:])
                r_pre = sb.tile([64, n], dt.float32, tag="hd_rpre")
                mm_bias_act(rW1_t[:], h3n, 64, rb1_t[:], r_pre)
                r_hid = sb.tile([64, n], dt.float32, tag="hd_rhid")
                _ln_relu_fm(nc, sb, ps, c, r_pre[:], n, rgm_t, rbe_t, r_hid[:], nfeat=64)

                rW2_t = sb.tile([64, 1], dt.float32, tag="hd_rW2")
                nc.sync.dma_start(out=rW2_t[:], in_=rW2[:])
                rb2_t = sb.tile([1, 1], dt.float32, tag="hd_rb2")
                nc.sync.dma_start(out=rb2_t[:], in_=rb2[:])
                rv = sb.tile([1, n], dt.float32, tag="hd_rv")
                for j in range(0, n, 512):
                    mm_ps = ps.tile([1, 512], dt.float32, space="PSUM", tag="pp_a")
                    nc.tensor.matmul(out=mm_ps[0:1, :], lhsT=rW2_t[:],
                                     rhs=r_hid[:, j : j + 512], start=True, stop=True)
                    nc.scalar.activation(out=rv[:, j : j + 512], in_=mm_ps[0:1, :],
                                         func=AF.Identity, bias=rb2_t[:], scale=1.0)
                # softplus then radius = 1 + 0.1 tanh(sp) = 1.1 - 0.2/(exp(2 sp)+1)
                sp = sb.tile([1, n], dt.float32, tag="hd_sp")
                nc.scalar.activation(out=sp[:], in_=rv[:], func=AF.Exp)
                nc.vector.tensor_scalar_add(sp[:], sp[:], 1.0)
                nc.scalar.activation(out=sp[:], in_=sp[:], func=AF.Ln)
                e2r = sb.tile([1, n], dt.float32, tag="hd_e2r")
                nc.scalar.activation(out=e2r[:], in_=sp[:], func=AF.Exp, scale=2.0)
                nc.vector.tensor_scalar_add(e2r[:], e2r[:], 1.0)
                rr2 = sb.tile([1, n], dt.float32, tag="hd_rr2")
                nc.vector.reciprocal(out=rr2[:], in_=e2r[:])
                radv = sb.tile([1, n], dt.float32, tag="hd_radv")
                nc.vector.tensor_scalar(out=radv[:], in0=rr2[:], scalar1=-0.2,
                                        scalar2=1.1, op0=OP.mult, op1=OP.add)
                nc.sync.dma_start(out=rad[:], in_=radv[:])
    nc.finalize()
    return nc


def build_p4():
    nc = bacc.Bacc(None, target_bir_lowering=False)
    ANG = nc.declare_dram_parameter("ANG", [P, 64], dt.float32, isOutput=False)
    RAD = nc.declare_dram_parameter("RAD", [P, 64], dt.float32, isOutput=False)
    CX = nc.declare_dram_parameter("CX", [P, 64], dt.float32, isOutput=True)
    CY = nc.declare_dram_parameter("CY", [P, 64], dt.float32, isOutput=True)
    with tile.TileContext(nc) as tc:
        with (
            tc.tile_pool(name="consts", bufs=1) as consts,
            tc.tile_pool(name="sb", bufs=1) as sb,
            tc.tile_pool(name="ps", bufs=1, space="PSUM") as ps,
        ):
            ones_col = consts.tile([P, 1], dt.float32)
            nc.gpsimd.memset(ones_col[:], 1.0)
            ones_row = consts.tile([1, P], dt.float32)
            nc.gpsimd.memset(ones_row[:], 1.0)
            half_pi = consts.tile([P, 1], dt.float32)
            nc.gpsimd.memset(half_pi[:], PI / 2.0)

            ang_t = sb.tile([P, 64], dt.float32)
            nc.sync.dma_start(out=ang_t[:], in_=ANG[:])
            rad_t = sb.tile([P, 64], dt.float32)
            nc.sync.dma_start(out=rad_t[:], in_=RAD[:])
            absang = sb.tile([P, 64], dt.float32)
            nc.scalar.activation(out=absang[:], in_=ang_t[:], func=AF.Abs)
            cosx = sb.tile([P, 64], dt.float32)
            nc.scalar.activation(out=cosx[:], in_=absang[:], func=AF.Sin,
                                 scale=-1.0, bias=half_pi[:])
            sinx = sb.tile([P, 64], dt.float32)
            nc.scalar.activation(out=sinx[:], in_=ang_t[:], func=AF.Sin)
            cx = sb.tile([P, 64], dt.float32)
            nc.vector.tensor_tensor(out=cx[:], in0=rad_t[:], in1=cosx[:], op=OP.mult)
            cy = sb.tile([P, 64], dt.float32)
            nc.vector.tensor_tensor(out=cy[:], in0=rad_t[:], in1=sinx[:], op=OP.mult)
            colsum = sb.tile([P, 2], dt.float32)
            nc.vector.tensor_reduce(out=colsum[:, 0:1], in_=cx[:],
                                    axis=mybir.AxisListType.X, op=OP.add)
            nc.vector.tensor_reduce(out=colsum[:, 1:2], in_=cy[:],
                                    axis=mybir.AxisListType.X, op=OP.add)
            tot_ps = ps.tile([1, 2], dt.float32, space="PSUM")
            nc.tensor.matmul(out=tot_ps[0:1, :], lhsT=ones_col[:], rhs=colsum[:],
                             start=True, stop=True)
            mean = sb.tile([1, 2], dt.float32)
            nc.vector.tensor_scalar_mul(mean[:], tot_ps[0:1, :], 1.0 / N)
            mean_rep = ps.tile([P, 2], dt.float32, space="PSUM")
            nc.tensor.matmul(out=mean_rep[:], lhsT=ones_row[:], rhs=mean[:],
                             start=True, stop=True)
            mrep_sb = sb.tile([P, 2], dt.float32)
            nc.vector.tensor_copy(out=mrep_sb[:], in_=mean_rep[:])
            nc.vector.tensor_tensor(out=cx[:], in0=cx[:],
                                    in1=mrep_sb[:, 0:1].to_broadcast([P, 64]),
                                    op=OP.subtract)
            nc.vector.tensor_tensor(out=cy[:], in0=cy[:],
                                    in1=mrep_sb[:, 1:2].to_broadcast([P, 64]),
                                    op=OP.subtract)
            q = sb.tile([P, 64], dt.float32)
            nc.vector.tensor_tensor(out=q[:], in0=cx[:], in1=cx[:], op=OP.mult)
            cy2 = sb.tile([P, 64], dt.float32)
            nc.vector.tensor_tensor(out=cy2[:], in0=cy[:], in1=cy[:], op=OP.mult)
            nc.vector.tensor_tensor(out=q[:], in0=q[:], in1=cy2[:], op=OP.add)
            nc.vector.tensor_scalar_max(q[:], q[:], 1e-24)
            # rsqrt: exp(-0.5 ln q) seed + one Newton polish (table accuracy)
            lnq = sb.tile([P, 64], dt.float32)
            nc.scalar.activation(out=lnq[:], in_=q[:], func=AF.Ln)
            y = sb.tile([P, 64], dt.float32)
            nc.scalar.activation(out=y[:], in_=lnq[:], func=AF.Exp, scale=-0.5)
            u = sb.tile([P, 64], dt.float32)
            for _ in range(2):
                nc.vector.tensor_tensor(out=u[:], in0=y[:], in1=y[:], op=OP.mult)
                nc.vector.tensor_tensor(out=u[:], in0=u[:], in1=q[:], op=OP.mult)
                nc.vector.tensor_scalar(out=u[:], in0=u[:], scalar1=-0.5, scalar2=1.5,
                                        op0=OP.mult, op1=OP.add)
                nc.vector.tensor_tensor(out=y[:], in0=y[:], in1=u[:], op=OP.mult)
            nc.vector.tensor_tensor(out=cx[:], in0=cx[:], in1=y[:], op=OP.mult)
            nc.vector.tensor_tensor(out=cy[:], in0=cy[:], in1=y[:], op=OP.mult)
            nc.sync.dma_start(out=CX[:], in_=cx[:])
            nc.sync.dma_start(out=CY[:], in_=cy[:])
    nc.finalize()
    return nc


# ----------------------------------------------------------------------------
# orchestration
# ----------------------------------------------------------------------------

_REP16 = np.zeros((8, P), np.float32)
for _h in range(8):
    _REP16[_h, _h * 16 : (_h + 1) * 16] = 1.0


def kernel(**inputs):
    from concourse.bass_utils import run_bass_kernel_spmd

    x = np.ascontiguousarray(np.asarray(inputs["x"], np.float32))
    traces = []

    def note(r):
        if r.instructions_and_trace:
            traces.append(r.instructions_and_trace[1])
        return r

    prep = host_prep(inputs["src"], inputs["dst"])
    order, K = prep["order"], prep["K"]
    cores = list(range(NCORES))

    xT = np.zeros((INP, N), np.float32)
    xT[:IN] = x[order].T
    W1p = np.zeros((INP, HC), np.float32)
    W1p[:IN] = np.asarray(inputs["W1"], np.float32)
    Mb = {l: mboth(np.asarray(inputs[f"as{l}"], np.float32),
                   np.asarray(inputs[f"ad{l}"], np.float32)) for l in (1, 2, 3)}
    cols = [core_cols(c) for c in cores]

    # ---- P1 ----
    p1 = build_p1()
    in_maps = [dict(xT=np.ascontiguousarray(xT[:, cols[c]]), W1=W1p, Mb=Mb[1])
               for c in cores]
    r1 = note(run_bass_kernel_spmd(p1, in_maps, cores))
    Tfull = np.zeros((N, 192), np.float32)
    for c in cores:
        Tfull[cols[c]] = r1.results[c]["Tout"]
    times = [r1.exec_time_ns]

    # ---- P2 (layers 2, 3) ----
    p2 = build_p23(K, with_next=True, with_head=False)
    xprev = [np.zeros((P, 1024), np.float32) for _ in cores]
    for l in (2, 3):
        in_maps = []
        for c in cores:
            in_maps.append(dict(
                Tfull=Tfull, Town=np.ascontiguousarray(Tfull[cols[c]]),
                xprev=xprev[c], idxq=prep["idxq"][c], maskq=prep["maskq"][c],
                bprev=np.asarray(inputs[f"b{l-1}"], np.float32).reshape(P, 1),
                gam=np.asarray(inputs[f"g{l-1}"], np.float32).reshape(P, 1),
                bet=np.asarray(inputs[f"be{l-1}"], np.float32).reshape(P, 1),
                Wn=np.ascontiguousarray(np.asarray(inputs[f"W{l}"], np.float32)),
                Mb=Mb[l], rep16q=_REP16,
            ))
        r2 = note(run_bass_kernel_spmd(p2, in_maps, cores))
        times.append(r2.exec_time_ns)
        Tn = np.zeros((N, 192), np.float32)
        for c in cores:
            Tn[cols[c]] = r2.results[c]["Tout"]
            xprev[c] = r2.results[c]["xnout"]
        Tfull = Tn

    # ---- P3 (layer-3 aggregation + MLP head) ----
    p3 = build_p23(K, with_next=False, with_head=True)
    in_maps = []
    for c in cores:
        in_maps.append(dict(
            Tfull=Tfull, Town=np.ascontiguousarray(Tfull[cols[c]]),
            xprev=xprev[c], idxq=prep["idxq"][c], maskq=prep["maskq"][c],
            bprev=np.asarray(inputs["b3"], np.float32).reshape(P, 1),
            gam=np.asarray(inputs["g3"], np.float32).reshape(P, 1),
            bet=np.asarray(inputs["be3"], np.float32).reshape(P, 1),
            rep16q=_REP16,
            aW1=np.ascontiguousarray(np.asarray(inputs["aW1"], np.float32)),
            ab1=np.asarray(inputs["ab1"], np.float32).reshape(P, 1),
            agm=np.asarray(inputs["ag"], np.float32).reshape(P, 1),
            abe=np.asarray(inputs["abe"], np.float32).reshape(P, 1),
            aW2=np.asarray(inputs["aW2"], np.float32).reshape(P, 1),
            ab2=np.asarray(inputs["ab2"], np.float32).reshape(1, 1),
            rW1=np.ascontiguousarray(np.asarray(inputs["rW1"], np.float32)),
            rb1=np.asarray(inputs["rb1"], np.float32).reshape(64, 1),
            rgm=np.asarray(inputs["rg"], np.float32).reshape(64, 1),
            rbe=np.asarray(inputs["rbe"], np.float32).reshape(64, 1),
            rW2=np.asarray(inputs["rW2"], np.float32).reshape(64, 1),
            rb2=np.asarray(inputs["rb2"], np.float32).reshape(1, 1),
        ))
    r3 = note(run_bass_kernel_spmd(p3, in_maps, cores))
    times.append(r3.exec_time_ns)
    ang = np.zeros(N, np.float32)
    rad = np.zeros(N, np.float32)
    for c in cores:
        ang[cols[c]] = r3.results[c]["ang"][0]
        rad[cols[c]] = r3.results[c]["rad"][0]

    # ---- P4 (finalize, replicated) ----
    p4 = build_p4()
    r4 = note(run_bass_kernel_spmd(
        p4, [dict(ANG=ang.reshape(P, 64), RAD=rad.reshape(P, 64))] * NCORES, cores))
    times.append(r4.exec_time_ns)
    cxv = r4.results[0]["CX"].reshape(N)
    cyv = r4.results[0]["CY"].reshape(N)

    out = np.zeros((N, 2), np.float32)
    out[order, 0] = cxv
    out[order, 1] = cyv
    kernel._last_times = times
    kernel._last_traces = traces
    return out



# revision 9
# speedup vs baseline: 1.4129x; 1.4129x over previous
"""Trainium2 Bass kernel for nn_GAT_86045374808682 (3-layer GAT + coordinate head).

Self-contained: takes FULL inputs, shards across 8 NeuronCores internally,
returns the FULL [8192, 2] float32 output.

v2 strategy (vs v1 fp32/768B-row baseline):
- Nodes relabeled by in-degree desc; 64 blocks of 128 striped across 8 cores
  (block j -> core j%8); per-stripe padded degree schedule K[t] (SPMD).
- Orthogonal per-head basis fold (U-trick): per head, rotate the 16-dim block
  by Q with first direction = a_src, scaled so the per-edge source score IS
  element h*16 of the row. Table row = 128 bf16 = 256 B (dma_gather minimum).
  Host folds Q*D into W (and its inverse B / a_dst' for the device).
- Gathers round-robin over 4 SWDGE queues (4x Q7 descriptor-prep parallelism),
  single_packet=False. Invalid slots gather a pad row (8192) whose score
  elements are -30000 -> exp()=0 (no mask tensor).
- Edge phase node-major: scores from strided slice, exp on ACT (only ACT table
  in P2), w = g*ex (DVE), k-sum via DVE reduce, den via reduce; per stripe:
  1/den normalize, one PE transpose, B-matmul back to h-basis, +bias.
- Batched LN over all 1024 cols: DVE square, Newton rsqrt (bit-trick seed),
  per-partition affine+relu on DVE. No ACT table swaps.
- P1: x/W1 in fp8 e4m3 (W1 pre-scaled x64, unscaled on PSUM copy-out).
- 5 launches: P1, P2 x2 (edge+node+pack), P3 (edge+head), P4 (trig finalize).
  Host concats slabs between launches (free in HW time).
"""
import sys

import numpy as np

for _p in ("/opt/trn_rl_repo", "/root/.axon_site/_ro/trn_rl_repo"):
    if _p not in sys.path:
        sys.path.append(_p)

import ml_dtypes

import concourse.bass as bass  # noqa: F401
import concourse.tile as tile
from concourse import bacc, library_config, mybir
from concourse.masks import make_identity

dt = mybir.dt
AF = mybir.ActivationFunctionType
OP = mybir.AluOpType

N = 8192
IN = 8193
INP = 8320  # 65 * 128
H = 8
HC = 128
P = 128
NCORES = 8
NSTRIPE = 8
KC = 16  # gather chunk (slots)
NQ = 4  # swdge queues
PAD_IDX = 8192
PADVAL = -30000.0
W1SCALE = 64.0
PI = float(np.pi)
MAGIC = 0x5F3759DF


# ----------------------------------------------------------------------------
# host-side prep
# ----------------------------------------------------------------------------

def host_prep(src, dst):
    s = np.concatenate([np.asarray(src).astype(np.int64), np.arange(N, dtype=np.int64)])
    d = np.concatenate([np.asarray(dst).astype(np.int64), np.arange(N, dtype=np.int64)])
    deg = np.bincount(d, minlength=N)
    order = np.argsort(-deg, kind="stable")  # new-id -> old-id
    old2new = np.empty(N, np.int64)
    old2new[order] = np.arange(N)
    s_new = old2new[s]
    d_new = old2new[d]
    deg_new = deg[order]

    K = [int(deg_new[1024 * t]) for t in range(NSTRIPE)]  # desc-sorted -> stripe max
    offs = np.cumsum([0] + K)

    eo = np.argsort(d_new, kind="stable")
    s_sorted = s_new[eo]
    starts = np.searchsorted(d_new[eo], np.arange(N))

    idxq = np.zeros((NCORES, 16, int(offs[-1]) * 8), np.int16)
    ar = np.arange(P)
    for c in range(NCORES):
        for t in range(NSTRIPE):
            Kt = K[t]
            vids = (t * NCORES + c) * P + ar
            e0 = starts[vids]
            degs = deg_new[vids]
            kk = np.arange(Kt)
            take = np.minimum(e0[:, None] + kk[None, :], len(s_sorted) - 1)
            mat = s_sorted[take]                      # [128, Kt]
            valid = kk[None, :] < degs[:, None]
            mat = np.where(valid, mat, PAD_IDX)
            lin = mat.T.reshape(-1)                   # slot-major [Kt*128]
            o16 = int(offs[t]) * 8
            idxq[c, :, o16 : o16 + Kt * 8] = lin.reshape(-1, 16).T
    return dict(order=order, K=K, offs=offs, idxq=idxq.astype(np.int16))


def core_cols(c):
    return np.concatenate([np.arange((t * NCORES + c) * P, (t * NCORES + c) * P + P)
                           for t in range(NSTRIPE)])


def fold_basis(a_src, a_dst):
    """Per-head orthogonal fold. Returns M [128,128] (fold into W: W' = W@M),
    B [128,128] (unfold: h = g@B), adp [128] (da = g . adp)."""
    M = np.zeros((HC, HC), np.float64)
    B = np.zeros((HC, HC), np.float64)
    adp = np.zeros(HC, np.float64)
    for h in range(H):
        a = np.asarray(a_src[h], np.float64)
        nrm = float(np.linalg.norm(a))
        j = int(np.argmax(np.abs(a))) if nrm > 0 else 0
        cols = [a if nrm > 0 else np.eye(16)[:, 0]]
        for i in range(16):
            if i != j:
                cols.append(np.eye(16)[:, i])
        A = np.stack(cols, axis=1)
        Q, R = np.linalg.qr(A)
        if R[0, 0] < 0:
            Q[:, 0] = -Q[:, 0]
        D = np.eye(16)
        D[0, 0] = nrm if nrm > 0 else 1.0
        Mh = Q @ D
        Bh = np.diag(1.0 / np.diag(D)) @ Q.T
        sl = slice(h * 16, (h + 1) * 16)
        M[sl, sl] = Mh
        B[sl, sl] = Bh
        adp[h * 16 : (h + 1) * 16] = Bh @ np.asarray(a_dst[h], np.float64)
    return M.astype(np.float32), B.astype(np.float32), adp.astype(np.float32)


def pad_row():
    r = np.zeros(HC, np.float32)
    for h in range(H):
        r[h * 16] = PADVAL
    return r


# ----------------------------------------------------------------------------
# device building blocks
# ----------------------------------------------------------------------------

def _newton_rsqrt(nc, sb, magic_t, v_ap, out_ap, w, tag, iters=2):
    """out = 1/sqrt(v) via magic-number seed + Newton. v > 0, fp32, [1, w]."""
    iv = sb.tile([1, 512], dt.int32, tag=f"{tag}_iv")
    nc.vector.tensor_scalar(out=iv[:, 0:w], in0=v_ap.bitcast(dt.int32), scalar1=1,
                            scalar2=None, op0=OP.logical_shift_right)
    nc.vector.tensor_tensor(out=iv[:, 0:w], in0=magic_t[:, 0:w], in1=iv[:, 0:w],
                            op=OP.subtract)
    y = iv.bitcast(dt.float32)
    u = sb.tile([1, 512], dt.float32, tag=f"{tag}_u")
    for _ in range(iters):
        nc.vector.tensor_tensor(out=u[:, 0:w], in0=y[:, 0:w], in1=y[:, 0:w], op=OP.mult)
        nc.vector.tensor_tensor(out=u[:, 0:w], in0=u[:, 0:w], in1=v_ap, op=OP.mult)
        nc.vector.tensor_scalar(out=u[:, 0:w], in0=u[:, 0:w], scalar1=-0.5,
                                scalar2=1.5, op0=OP.mult, op1=OP.add)
        nc.vector.tensor_tensor(out=y[:, 0:w], in0=y[:, 0:w], in1=u[:, 0:w], op=OP.mult)
    nc.vector.tensor_copy(out=out_ap, in_=y[:, 0:w])


def _ln_relu_fm(nc, sb, ps, c, x_sb, n, gam_t, bet_t, out_sb, nfeat=P, eps=1e-5):
    """Feature-major LN+affine+ReLU, DVE-only elementwise (no ACT tables)."""
    for j in range(0, n, 512):
        w = min(512, n - j)
        xs = x_sb[:, j : j + w]
        xsq = sb.tile([nfeat, 512], dt.float32, tag="ln_xsq")
        nc.vector.tensor_tensor(out=xsq[:, 0:w], in0=xs, in1=xs, op=OP.mult)
        s1_ps = ps.tile([1, 512], dt.float32, space="PSUM", tag="pp_a")
        nc.tensor.matmul(out=s1_ps[:, 0:w], lhsT=c["ones_col"][0:nfeat, :], rhs=xs,
                         start=True, stop=True)
        s2_ps = ps.tile([1, 512], dt.float32, space="PSUM", tag="pp_b")
        nc.tensor.matmul(out=s2_ps[:, 0:w], lhsT=c["ones_col"][0:nfeat, :],
                         rhs=xsq[:, 0:w], start=True, stop=True)
        mu = sb.tile([1, 512], dt.float32, tag="ln_mu")
        nc.vector.tensor_scalar_mul(mu[:, 0:w], s1_ps[:, 0:w], 1.0 / nfeat)
        musq = sb.tile([1, 512], dt.float32, tag="ln_musq")
        nc.vector.tensor_tensor(out=musq[:, 0:w], in0=mu[:, 0:w], in1=mu[:, 0:w],
                                op=OP.mult)
        var = sb.tile([1, 512], dt.float32, tag="ln_var")
        nc.vector.scalar_tensor_tensor(out=var[:, 0:w], in0=s2_ps[:, 0:w],
                                       scalar=1.0 / nfeat, in1=musq[:, 0:w],
                                       op0=OP.mult, op1=OP.subtract)
        nc.vector.tensor_scalar_add(var[:, 0:w], var[:, 0:w], float(eps))
        rs = sb.tile([1, 512], dt.float32, tag="ln_rs")
        _newton_rsqrt(nc, sb, c["magic"], var[:, 0:w], rs[:, 0:w], w, "ln")
        rep_mu = ps.tile([nfeat, 512], dt.float32, space="PSUM", tag="pp_a")
        nc.tensor.matmul(out=rep_mu[:, 0:w], lhsT=c["ones_row"][:, 0:nfeat],
                         rhs=mu[:, 0:w], start=True, stop=True)
        rep_rs = ps.tile([nfeat, 512], dt.float32, space="PSUM", tag="pp_b")
        nc.tensor.matmul(out=rep_rs[:, 0:w], lhsT=c["ones_row"][:, 0:nfeat],
                         rhs=rs[:, 0:w], start=True, stop=True)
        xh = sb.tile([nfeat, 512], dt.float32, tag="ln_xh")
        nc.vector.tensor_tensor(out=xh[:, 0:w], in0=xs, in1=rep_mu[:, 0:w],
                                op=OP.subtract)
        nc.vector.tensor_tensor(out=xh[:, 0:w], in0=xh[:, 0:w], in1=rep_rs[:, 0:w],
                                op=OP.mult)
        nc.vector.tensor_scalar(out=xh[:, 0:w], in0=xh[:, 0:w], scalar1=gam_t[:],
                                scalar2=bet_t[:], op0=OP.mult, op1=OP.add)
        nc.vector.tensor_scalar_max(out_sb[:, j : j + w], xh[:, 0:w], 0.0)


def _mk_consts(nc, consts, need_gather=True):
    c = {}
    if need_gather:
        nc.gpsimd.load_library(library_config.mlp)
    c["ident"] = consts.tile([P, P], dt.bfloat16, name="c_ident")
    make_identity(nc, c["ident"][:])
    c["ones_col"] = consts.tile([P, 1], dt.float32, name="c_ones_col")
    nc.gpsimd.memset(c["ones_col"][:], 1.0)
    c["ones_row"] = consts.tile([1, P], dt.float32, name="c_ones_row")
    nc.gpsimd.memset(c["ones_row"][:], 1.0)
    c["magic"] = consts.tile([1, 512], dt.int32, name="c_magic")
    nc.gpsimd.memset(c["magic"][:], MAGIC)
    return c


# ----------------------------------------------------------------------------
# P1: g1 = (x @ W1') in fp8, pack bf16 rows
# ----------------------------------------------------------------------------

def build_p1():
    nc = bacc.Bacc(None, target_bir_lowering=False)
    xT = nc.declare_dram_parameter("xT", [INP, 1024], dt.bfloat16, isOutput=False)
    W1 = nc.declare_dram_parameter("W1", [INP, HC], dt.bfloat16, isOutput=False)
    Tout = nc.declare_dram_parameter("Tout", [1024, HC], dt.bfloat16, isOutput=True)

    with tile.TileContext(nc) as tc:
        with (
            tc.tile_pool(name="consts", bufs=1) as consts,
            tc.tile_pool(name="xp", bufs=4) as xp,
            tc.tile_pool(name="sb", bufs=2) as sb,
            tc.tile_pool(name="psh", bufs=2, space="PSUM") as psh,
            tc.tile_pool(name="ps", bufs=2, space="PSUM") as ps,
        ):
            ident = consts.tile([P, P], dt.bfloat16)
            make_identity(nc, ident[:])
            w1_t = consts.tile([P, 65, P], dt.bfloat16)
            nc.sync.dma_start(out=w1_t[:], in_=W1.rearrange("(k p) f -> p k f", p=P))
            for nb in range(2):
                hps = psh.tile([P, 512], dt.float32, space="PSUM", tag="hps")
                for kcb in range(65):
                    xt = xp.tile([P, 512], dt.bfloat16, tag="xt")
                    nc.sync.dma_start(out=xt[:], in_=xT[kcb * P : (kcb + 1) * P,
                                                        nb * 512 : (nb + 1) * 512])
                    nc.tensor.matmul(out=hps[:], lhsT=w1_t[:, kcb, :], rhs=xt[:],
                                     start=(kcb == 0), stop=(kcb == 64))
                h_sb = sb.tile([P, 512], dt.bfloat16, tag="h_sb")
                nc.vector.tensor_copy(out=h_sb[:], in_=hps[:])
                for b in range(4):
                    blk = nb * 4 + b
                    ht_ps = ps.tile([P, P], dt.bfloat16, space="PSUM", tag="pp_t")
                    nc.tensor.matmul(out=ht_ps[:], lhsT=h_sb[:, b * P : (b + 1) * P],
                                     rhs=ident[:], is_transpose=True, start=True,
                                     stop=True)
                    pk = sb.tile([P, P], dt.bfloat16, tag="pk")
                    nc.vector.tensor_copy(out=pk[:], in_=ht_ps[:])
                    nc.sync.dma_start(out=Tout[blk * P : (blk + 1) * P, :], in_=pk[:])
    nc.finalize()
    return nc


# ----------------------------------------------------------------------------
# P2/P3: edge aggregation + node phase (+ pack next | + MLP head)
# ----------------------------------------------------------------------------

def build_p23(K, with_next, with_head):
    SK = int(sum(K))
    offs = np.cumsum([0] + list(K))
    nc = bacc.Bacc(None, target_bir_lowering=False, num_swdge_queues=NQ)
    Tfull = nc.declare_dram_parameter("Tfull", [INP, HC], dt.bfloat16, isOutput=False)
    Town = nc.declare_dram_parameter("Town", [1024, HC], dt.bfloat16, isOutput=False)
    xprev = nc.declare_dram_parameter("xprev", [P, 1024], dt.float32, isOutput=False)
    idxq = nc.declare_dram_parameter("idxq", [16, SK * 8], dt.int16, isOutput=False)
    bprev = nc.declare_dram_parameter("bprev", [P, 1], dt.float32, isOutput=False)
    gam = nc.declare_dram_parameter("gam", [P, 1], dt.float32, isOutput=False)
    bet = nc.declare_dram_parameter("bet", [P, 1], dt.float32, isOutput=False)
    adpr = nc.declare_dram_parameter("adpr", [P, HC], dt.bfloat16, isOutput=False)
    Bmat = nc.declare_dram_parameter("Bmat", [P, P], dt.bfloat16, isOutput=False)
    rep16q = nc.declare_dram_parameter("rep16q", [H, P], dt.float32, isOutput=False)
    if with_next:
        Wn = nc.declare_dram_parameter("Wn", [P, P], dt.float32, isOutput=False)
        Tout = nc.declare_dram_parameter("Tout", [1024, HC], dt.bfloat16, isOutput=True)
        xnout = nc.declare_dram_parameter("xnout", [P, 1024], dt.float32, isOutput=True)
    if with_head:
        aW1 = nc.declare_dram_parameter("aW1", [P, P], dt.float32, isOutput=False)
        ab1 = nc.declare_dram_parameter("ab1", [P, 1], dt.float32, isOutput=False)
        agm = nc.declare_dram_parameter("agm", [P, 1], dt.float32, isOutput=False)
        abe = nc.declare_dram_parameter("abe", [P, 1], dt.float32, isOutput=False)
        aW2 = nc.declare_dram_parameter("aW2", [P, 1], dt.float32, isOutput=False)
        ab2 = nc.declare_dram_parameter("ab2", [1, 1], dt.float32, isOutput=False)
        rW1 = nc.declare_dram_parameter("rW1", [P, 64], dt.float32, isOutput=False)
        rb1 = nc.declare_dram_parameter("rb1", [64, 1], dt.float32, isOutput=False)
        rgm = nc.declare_dram_parameter("rgm", [64, 1], dt.float32, isOutput=False)
        rbe = nc.declare_dram_parameter("rbe", [64, 1], dt.float32, isOutput=False)
        rW2 = nc.declare_dram_parameter("rW2", [64, 1], dt.float32, isOutput=False)
        rb2 = nc.declare_dram_parameter("rb2", [1, 1], dt.float32, isOutput=False)
        ang = nc.declare_dram_parameter("ang", [1, 1024], dt.float32, isOutput=True)
        rad = nc.declare_dram_parameter("rad", [1, 1024], dt.float32, isOutput=True)

    with tile.TileContext(nc) as tc:
        with (
            tc.tile_pool(name="consts", bufs=1) as consts,
            tc.tile_pool(name="gpool", bufs=5) as gpool,
            tc.tile_pool(name="wpool", bufs=3) as wpool,
            tc.tile_pool(name="sc", bufs=3) as sc,
            tc.tile_pool(name="sb", bufs=1) as sb,
            tc.tile_pool(name="ps", bufs=2, space="PSUM") as ps,
            tc.tile_pool(name="pst", bufs=1, space="PSUM") as pst,
            tc.tile_pool(name="psagg", bufs=2, space="PSUM") as psagg,
        ):
            c = _mk_consts(nc, consts)
            c["identf"] = consts.tile([P, P], dt.float32, name="c_identf")
            make_identity(nc, c["identf"][:])
            rep16_t = consts.tile([H, P], dt.float32, name="c_rep16")
            nc.sync.dma_start(out=rep16_t[:], in_=rep16q[:])

            idx_t = sb.tile([P, SK * 8], dt.int16, tag="idx")
            for a in range(8):
                nc.sync.dma_start(out=idx_t[16 * a : 16 * (a + 1), :], in_=idxq[:])
            town_t = sb.tile([P, NSTRIPE, HC], dt.bfloat16, tag="town")
            nc.sync.dma_start(out=town_t[:],
                              in_=Town.rearrange("(t p) f -> p t f", p=P))
            xprev_t = sb.tile([P, 1024], dt.float32, tag="xprev")
            nc.sync.dma_start(out=xprev_t[:], in_=xprev[:])
            adpr_t = sb.tile([P, HC], dt.bfloat16, tag="adpr")
            nc.sync.dma_start(out=adpr_t[:], in_=adpr[:])
            bias_t = sb.tile([P, 1], dt.float32, tag="bias")
            nc.sync.dma_start(out=bias_t[:], in_=bprev[:])
            gam_t = sb.tile([P, 1], dt.float32, tag="gam")
            nc.sync.dma_start(out=gam_t[:], in_=gam[:])
            bet_t = sb.tile([P, 1], dt.float32, tag="bet")
            nc.sync.dma_start(out=bet_t[:], in_=bet[:])
            bmat_t = sb.tile([P, P], dt.bfloat16, tag="bmat")
            nc.sync.dma_start(out=bmat_t[:], in_=Bmat[:])
            if with_next:
                wn_t = sb.tile([P, P], dt.float32, tag="wn")
                nc.sync.dma_start(out=wn_t[:], in_=Wn[:])

            # da[p, t, h] = sum_j town[p, t, h*16+j] * adp[h*16+j]
            dam = sb.tile([P, NSTRIPE, HC], dt.bfloat16, tag="dam")
            nc.vector.tensor_tensor(
                out=dam[:], in0=town_t[:],
                in1=adpr_t.unsqueeze(1).to_broadcast([P, NSTRIPE, HC]), op=OP.mult)
            da_t = sb.tile([P, NSTRIPE, H], dt.float32, tag="da")
            nc.vector.tensor_reduce(
                out=da_t[:], in_=dam[:].rearrange("p t (h j) -> p t h j", j=16),
                axis=mybir.AxisListType.X, op=OP.add)

            xball = sb.tile([P, 1024], dt.float32, tag="xball")
            gq = [0]

            for t in range(NSTRIPE):
                K_t, off_t = K[t], int(offs[t])
                nchunk = (K_t + KC - 1) // KC
                agg_ps = psagg.tile([P, P], dt.float32, space="PSUM", tag="agg")
                den = sc.tile([P, H], dt.float32, tag="den")
                for ci in range(nchunk):
                    k0 = ci * KC
                    kc = min(KC, K_t - k0)
                    g = gpool.tile([P, KC, HC], dt.bfloat16, tag="g")
                    nc.gpsimd.dma_gather(
                        out_ap=g[:, 0:kc, :],
                        in_ap=Tfull[:],
                        idxs_ap=idx_t[:, (off_t + k0) * 8 : (off_t + k0 + kc) * 8],
                        num_idxs=kc * P,
                        num_idxs_reg=kc * P,
                        elem_size=HC,
                        single_packet=False,
                        queue_num=gq[0] % NQ,
                    )
                    gq[0] += 1
                    g4 = g[:, 0:kc, :].rearrange("p k (h j) -> p k h j", j=16)
                    z = sc.tile([P, KC, H], dt.float32, tag="z")
                    nc.vector.tensor_tensor(
                        out=z[:, 0:kc, :],
                        in0=g4[:, :, :, 0:1].rearrange("p k h j -> p k (h j)"),
                        in1=da_t[:, t, :].unsqueeze(1).to_broadcast([P, kc, H]),
                        op=OP.add)
                    zl = sc.tile([P, KC, H], dt.float32, tag="zl")
                    nc.vector.scalar_tensor_tensor(out=zl[:, 0:kc, :], in0=z[:, 0:kc, :],
                                                   scalar=0.2, in1=z[:, 0:kc, :],
                                                   op0=OP.mult, op1=OP.max)
                    ex8 = sc.tile([P, KC, H], dt.bfloat16, tag="ex8")
                    nc.scalar.activation(out=ex8[:, 0:kc, :], in_=zl[:, 0:kc, :],
                                         func=AF.Exp)
                    dc = sc.tile([P, H], dt.float32, tag="dc")
                    nc.vector.tensor_reduce(out=dc[:],
                                            in_=ex8[:, 0:kc, :].transpose([0, 2, 1]),
                                            axis=mybir.AxisListType.X, op=OP.add)
                    if ci == 0:
                        nc.vector.tensor_copy(out=den[:], in_=dc[:])
                    else:
                        nc.vector.tensor_tensor(out=den[:], in0=den[:], in1=dc[:],
                                                op=OP.add)
                    exr = wpool.tile([P, KC, HC], dt.bfloat16, tag="exr")
                    nc.scalar.activation(
                        out=exr[:, 0:kc, :].rearrange("p k (h j) -> p k h j", j=16),
                        in_=zl[:, 0:kc, :].unsqueeze(3).to_broadcast([P, kc, H, 16]),
                        func=AF.Exp)
                    w = wpool.tile([P, KC, HC], dt.bfloat16, tag="w")
                    nc.vector.tensor_tensor(
                        out=w[:, 0:kc, :].rearrange("p k f -> p (k f)"),
                        in0=g[:, 0:kc, :].rearrange("p k f -> p (k f)"),
                        in1=exr[:, 0:kc, :].rearrange("p k f -> p (k f)"),
                        op=OP.mult)
                    for k in range(kc):
                        nc.tensor.matmul(out=agg_ps[:], lhsT=w[:, k, :],
                                         rhs=c["ident"][:],
                                         start=(ci == 0 and k == 0),
                                         stop=(ci == nchunk - 1 and k == kc - 1))
                dent = pst.tile([H, P], dt.float32, space="PSUM", tag="pp_d")
                nc.tensor.matmul(out=dent[0:H, :], lhsT=den[:], rhs=c["identf"][:],
                                 start=True, stop=True)
                rden = sc.tile([H, P], dt.float32, tag="rden")
                nc.vector.reciprocal(out=rden[:], in_=dent[0:H, :])
                rdrep = pst.tile([P, P], dt.float32, space="PSUM", tag="pp_d")
                nc.tensor.matmul(out=rdrep[:], lhsT=rep16_t[:], rhs=rden[:],
                                 start=True, stop=True)
                rdsb = sc.tile([P, P], dt.float32, tag="rdsb")
                nc.vector.tensor_copy(out=rdsb[:], in_=rdrep[:])
                aggn = sc.tile([P, P], dt.bfloat16, tag="aggn")
                nc.vector.tensor_tensor(out=aggn[:], in0=agg_ps[:], in1=rdsb[:],
                                        op=OP.mult)
                hps = pst.tile([P, P], dt.float32, space="PSUM", tag="pp_h")
                nc.tensor.matmul(out=hps[:], lhsT=bmat_t[:], rhs=aggn[:],
                                 start=True, stop=True)
                nc.vector.tensor_scalar_add(xball[:, t * P : (t + 1) * P], hps[:],
                                            bias_t[:])

            # node phase, batched over all 1024 columns
            xo = sb.tile([P, 1024], dt.float32, tag="xo")
            _ln_relu_fm(nc, sb, ps, c, xball[:], 1024, gam_t, bet_t, xo[:])
            xnext = sb.tile([P, 1024], dt.float32, tag="xnext")
            nc.vector.tensor_tensor(out=xnext[:], in0=xo[:], in1=xprev_t[:], op=OP.add)

            if with_next:
                for j in range(0, 1024, 512):
                    gps = ps.tile([P, 512], dt.float32, space="PSUM", tag="pp_a")
                    nc.tensor.matmul(out=gps[:], lhsT=wn_t[:],
                                     rhs=xnext[:, j : j + 512], start=True, stop=True)
                    gsb = sb.tile([P, 512], dt.bfloat16, tag="gsb")
                    nc.vector.tensor_copy(out=gsb[:], in_=gps[:])
                    for b in range(4):
                        blk = j // P + b
                        tp2 = pst.tile([P, P], dt.bfloat16, space="PSUM", tag="pp_h")
                        nc.tensor.matmul(out=tp2[:], lhsT=gsb[:, b * P : (b + 1) * P],
                                         rhs=c["ident"][:], is_transpose=True,
                                         start=True, stop=True)
                        pk = sb.tile([P, P], dt.bfloat16, tag="pk")
                        nc.vector.tensor_copy(out=pk[:], in_=tp2[:])
                        nc.sync.dma_start(out=Tout[blk * P : (blk + 1) * P, :],
                                          in_=pk[:])
                nc.sync.dma_start(out=xnout[:], in_=xnext[:])

            if with_head:
                n = 1024
                # h3n = xnext / row_norm
                xsq = sb.tile([P, n], dt.float32, tag="hd_xsq")
                nc.vector.tensor_tensor(out=xsq[:], in0=xnext[:], in1=xnext[:],
                                        op=OP.mult)
                h3n = sb.tile([P, n], dt.float32, tag="hd_h3n")
                for j in range(0, n, 512):
                    ss_ps = ps.tile([1, 512], dt.float32, space="PSUM", tag="pp_a")
                    nc.tensor.matmul(out=ss_ps[0:1, :], lhsT=c["ones_col"][:],
                                     rhs=xsq[:, j : j + 512], start=True, stop=True)
                    ss = sb.tile([1, 512], dt.float32, tag="hd_ss")
                    nc.vector.tensor_scalar_max(ss[:], ss_ps[0:1, :], 1e-24)
                    rn = sb.tile([1, 512], dt.float32, tag="hd_rn")
                    _newton_rsqrt(nc, sb, c["magic"], ss[:], rn[:], 512, "hd", iters=3)
                    rn_rep = ps.tile([P, 512], dt.float32, space="PSUM", tag="pp_b")
                    nc.tensor.matmul(out=rn_rep[:], lhsT=c["ones_row"][:], rhs=rn[:],
                                     start=True, stop=True)
                    nc.vector.tensor_tensor(out=h3n[:, j : j + 512],
                                            in0=xnext[:, j : j + 512],
                                            in1=rn_rep[:], op=OP.mult)

                def mm_bias(lhsT_t, rhs_sb, m, bias_ap, out_sb):
                    for j in range(0, n, 512):
                        mm_ps = ps.tile([P, 512], dt.float32, space="PSUM", tag="pp_a")
                        nc.tensor.matmul(out=mm_ps[0:m, :], lhsT=lhsT_t,
                                         rhs=rhs_sb[:, j : j + 512], start=True,
                                         stop=True)
                        nc.vector.tensor_scalar_add(out_sb[0:m, j : j + 512],
                                                    mm_ps[0:m, :], bias_ap)

                aW1_t = sb.tile([P, P], dt.float32, tag="hd_aW1")
                nc.sync.dma_start(out=aW1_t[:], in_=aW1[:])
                ab1_t = sb.tile([P, 1], dt.float32, tag="hd_ab1")
                nc.sync.dma_start(out=ab1_t[:], in_=ab1[:])
                agm_t = sb.tile([P, 1], dt.float32, tag="hd_agm")
                nc.sync.dma_start(out=agm_t[:], in_=agm[:])
                abe_t = sb.tile([P, 1], dt.float32, tag="hd_abe")
                nc.sync.dma_start(out=abe_t[:], in_=abe[:])
                a_pre = sb.tile([P, n], dt.float32, tag="hd_apre")
                mm_bias(aW1_t[:], h3n, P, ab1_t[:], a_pre)
                a_hid = sb.tile([P, n], dt.float32, tag="hd_ahid")
                _ln_relu_fm(nc, sb, ps, c, a_pre[:], n, agm_t, abe_t, a_hid[:])

                aW2_t = sb.tile([P, 1], dt.float32, tag="hd_aW2")
                nc.sync.dma_start(out=aW2_t[:], in_=aW2[:])
                ab2_t = sb.tile([1, 1], dt.float32, tag="hd_ab2")
                nc.sync.dma_start(out=ab2_t[:], in_=ab2[:])
                av = sb.tile([1, n], dt.float32, tag="hd_av")
                mm_bias(aW2_t[:], a_hid, 1, ab2_t[:], av)
                # angles = pi*tanh(av) = pi - 2pi/(exp(2av)+1)
                e2 = sb.tile([1, n], dt.float32, tag="hd_e2")
                nc.scalar.activation(out=e2[:], in_=av[:], func=AF.Exp, scale=2.0)
                nc.vector.tensor_scalar_add(e2[:], e2[:], 1.0)
                rr = sb.tile([1, n], dt.float32, tag="hd_rr")
                nc.vector.reciprocal(out=rr[:], in_=e2[:])
                angv = sb.tile([1, n], dt.float32, tag="hd_angv")
                nc.vector.tensor_scalar(out=angv[:], in0=rr[:], scalar1=-2.0 * PI,
                                        scalar2=PI, op0=OP.mult, op1=OP.add)
                nc.sync.dma_start(out=ang[:], in_=angv[:])

                rW1_t = sb.tile([P, 64], dt.float32, tag="hd_rW1")
                nc.sync.dma_start(out=rW1_t[:], in_=rW1[:])
                rb1_t = sb.tile([64, 1], dt.float32, tag="hd_rb1")
                nc.sync.dma_start(out=rb1_t[:], in_=rb1[:])
                rgm_t = sb.tile([64, 1], dt.float32, tag="hd_rgm")
                nc.sync.dma_start(out=rgm_t[:], in_=rgm[:])
                rbe_t = sb.tile([64, 1], dt.float32, tag="hd_rbe")
                nc.sync.dma_start(out=rbe_t[:], in_=rbe[:])
                r_pre = sb.tile([64, n], dt.float32, tag="hd_rpre")
                mm_bias(rW1_t[:], h3n, 64, rb1_t[:], r_pre)
                r_hid = sb.tile([64, n], dt.float32, tag="hd_rhid")
                _ln_relu_fm(nc, sb, ps, c, r_pre[:], n, rgm_t, rbe_t, r_hid[:],
                            nfeat=64)

                rW2_t = sb.tile([64, 1], dt.float32, tag="hd_rW2")
                nc.sync.dma_start(out=rW2_t[:], in_=rW2[:])
                rb2_t = sb.tile([1, 1], dt.float32, tag="hd_rb2")
                nc.sync.dma_start(out=rb2_t[:], in_=rb2[:])
                rv = sb.tile([1, n], dt.float32, tag="hd_rv")
                for j in range(0, n, 512):
                    mm_ps = ps.tile([1, 512], dt.float32, space="PSUM", tag="pp_a")
                    nc.tensor.matmul(out=mm_ps[0:1, :], lhsT=rW2_t[:],
                                     rhs=r_hid[:, j : j + 512], start=True, stop=True)
                    nc.vector.tensor_scalar_add(rv[:, j : j + 512], mm_ps[0:1, :],
                                                rb2_t[:])
                # softplus then radius = 1 + 0.1 tanh(sp) = 1.1 - 0.2/(exp(2 sp)+1)
                sp = sb.tile([1, n], dt.float32, tag="hd_sp")
                nc.scalar.activation(out=sp[:], in_=rv[:], func=AF.Exp)
                nc.vector.tensor_scalar_add(sp[:], sp[:], 1.0)
                nc.scalar.activation(out=sp[:], in_=sp[:], func=AF.Ln)
                e2r = sb.tile([1, n], dt.float32, tag="hd_e2r")
                nc.scalar.activation(out=e2r[:], in_=sp[:], func=AF.Exp, scale=2.0)
                nc.vector.tensor_scalar_add(e2r[:], e2r[:], 1.0)
                rr2 = sb.tile([1, n], dt.float32, tag="hd_rr2")
                nc.vector.reciprocal(out=rr2[:], in_=e2r[:])
                radv = sb.tile([1, n], dt.float32, tag="hd_radv")
                nc.vector.tensor_scalar(out=radv[:], in0=rr2[:], scalar1=-0.2,
                                        scalar2=1.1, op0=OP.mult, op1=OP.add)
                nc.sync.dma_start(out=rad[:], in_=radv[:])
    nc.finalize()
    return nc


# ----------------------------------------------------------------------------
# P4: trig finalize (replicated)
# ----------------------------------------------------------------------------

def build_p4():
    nc = bacc.Bacc(None, target_bir_lowering=False)
    ANG = nc.declare_dram_parameter("ANG", [P, 64], dt.float32, isOutput=False)
    RAD = nc.declare_dram_parameter("RAD", [P, 64], dt.float32, isOutput=False)
    CX = nc.declare_dram_parameter("CX", [P, 64], dt.float32, isOutput=True)
    CY = nc.declare_dram_parameter("CY", [P, 64], dt.float32, isOutput=True)
    with tile.TileContext(nc) as tc:
        with (
            tc.tile_pool(name="consts", bufs=1) as consts,
            tc.tile_pool(name="sb", bufs=1) as sb,
            tc.tile_pool(name="ps", bufs=1, space="PSUM") as ps,
        ):
            ones_col = consts.tile([P, 1], dt.float32)
            nc.gpsimd.memset(ones_col[:], 1.0)
            ones_row = consts.tile([1, P], dt.float32)
            nc.gpsimd.memset(ones_row[:], 1.0)
            half_pi = consts.tile([P, 1], dt.float32)
            nc.gpsimd.memset(half_pi[:], PI / 2.0)
            magic = consts.tile([P, 64], dt.int32)
            nc.gpsimd.memset(magic[:], MAGIC)

            ang_t = sb.tile([P, 64], dt.float32)
            nc.sync.dma_start(out=ang_t[:], in_=ANG[:])
            rad_t = sb.tile([P, 64], dt.float32)
            nc.sync.dma_start(out=rad_t[:], in_=RAD[:])
            absang = sb.tile([P, 64], dt.float32)
            nc.vector.scalar_tensor_tensor(out=absang[:], in0=ang_t[:], scalar=-1.0,
                                           in1=ang_t[:], op0=OP.mult, op1=OP.max)
            cosx = sb.tile([P, 64], dt.float32)
            nc.scalar.activation(out=cosx[:], in_=absang[:], func=AF.Sin,
                                 scale=-1.0, bias=half_pi[:])
            sinx = sb.tile([P, 64], dt.float32)
            nc.scalar.activation(out=sinx[:], in_=ang_t[:], func=AF.Sin)
            cx = sb.tile([P, 64], dt.float32)
            nc.vector.tensor_tensor(out=cx[:], in0=rad_t[:], in1=cosx[:], op=OP.mult)
            cy = sb.tile([P, 64], dt.float32)
            nc.vector.tensor_tensor(out=cy[:], in0=rad_t[:], in1=sinx[:], op=OP.mult)
            colsum = sb.tile([P, 2], dt.float32)
            nc.vector.tensor_reduce(out=colsum[:, 0:1], in_=cx[:],
                                    axis=mybir.AxisListType.X, op=OP.add)
            nc.vector.tensor_reduce(out=colsum[:, 1:2], in_=cy[:],
                                    axis=mybir.AxisListType.X, op=OP.add)
            tot_ps = ps.tile([1, 2], dt.float32, space="PSUM")
            nc.tensor.matmul(out=tot_ps[0:1, :], lhsT=ones_col[:], rhs=colsum[:],
                             start=True, stop=True)
            mean = sb.tile([1, 2], dt.float32)
            nc.vector.tensor_scalar_mul(mean[:], tot_ps[0:1, :], 1.0 / N)
            mean_rep = ps.tile([P, 2], dt.float32, space="PSUM")
            nc.tensor.matmul(out=mean_rep[:], lhsT=ones_row[:], rhs=mean[:],
                             start=True, stop=True)
            mrep_sb = sb.tile([P, 2], dt.float32)
            nc.vector.tensor_copy(out=mrep_sb[:], in_=mean_rep[:])
            nc.vector.tensor_tensor(out=cx[:], in0=cx[:],
                                    in1=mrep_sb[:, 0:1].to_broadcast([P, 64]),
                                    op=OP.subtract)
            nc.vector.tensor_tensor(out=cy[:], in0=cy[:],
                                    in1=mrep_sb[:, 1:2].to_broadcast([P, 64]),
                                    op=OP.subtract)
            q = sb.tile([P, 64], dt.float32)
            nc.vector.tensor_tensor(out=q[:], in0=cx[:], in1=cx[:], op=OP.mult)
            cy2 = sb.tile([P, 64], dt.float32)
            nc.vector.tensor_tensor(out=cy2[:], in0=cy[:], in1=cy[:], op=OP.mult)
            nc.vector.tensor_tensor(out=q[:], in0=q[:], in1=cy2[:], op=OP.add)
            nc.vector.tensor_scalar_max(q[:], q[:], 1e-24)
            iv = sb.tile([P, 64], dt.int32)
            nc.vector.tensor_scalar(out=iv[:], in0=q[:].bitcast(dt.int32), scalar1=1,
                                    scalar2=None, op0=OP.logical_shift_right)
            nc.vector.tensor_tensor(out=iv[:], in0=magic[:], in1=iv[:], op=OP.subtract)
            y = iv.bitcast(dt.float32)
            u = sb.tile([P, 64], dt.float32)
            for _ in range(3):
                nc.vector.tensor_tensor(out=u[:], in0=y[:], in1=y[:], op=OP.mult)
                nc.vector.tensor_tensor(out=u[:], in0=u[:], in1=q[:], op=OP.mult)
                nc.vector.tensor_scalar(out=u[:], in0=u[:], scalar1=-0.5, scalar2=1.5,
                                        op0=OP.mult, op1=OP.add)
                nc.vector.tensor_tensor(out=y[:], in0=y[:], in1=u[:], op=OP.mult)
            nc.vector.tensor_tensor(out=cx[:], in0=cx[:], in1=y[:], op=OP.mult)
            nc.vector.tensor_tensor(out=cy[:], in0=cy[:], in1=y[:], op=OP.mult)
            nc.sync.dma_start(out=CX[:], in_=cx[:])
            nc.sync.dma_start(out=CY[:], in_=cy[:])
    nc.finalize()
    return nc


# ----------------------------------------------------------------------------
# orchestration
# ----------------------------------------------------------------------------

_REP16 = np.zeros((H, P), np.float32)
for _h in range(H):
    _REP16[_h, _h * 16 : (_h + 1) * 16] = 1.0


def kernel(**inputs):
    from concourse.bass_utils import run_bass_kernel_spmd

    bf16 = ml_dtypes.bfloat16
    f8 = ml_dtypes.float8_e4m3fn

    x = np.ascontiguousarray(np.asarray(inputs["x"], np.float32))
    traces = []

    def note(r):
        if r.instructions_and_trace:
            traces.append(r.instructions_and_trace[1])
        return r

    prep = host_prep(inputs["src"], inputs["dst"])
    order, K = prep["order"], prep["K"]
    cores = list(range(NCORES))
    cols = [core_cols(c) for c in cores]

    # per-layer basis folds
    M = {}
    B = {}
    ADP = {}
    for l in (1, 2, 3):
        M[l], B[l], ADP[l] = fold_basis(np.asarray(inputs[f"as{l}"], np.float32),
                                        np.asarray(inputs[f"ad{l}"], np.float32))

    xT = np.zeros((INP, N), np.float32)
    xT[:IN] = x[order].T
    W1f = np.zeros((INP, HC), np.float32)
    W1f[:IN] = np.asarray(inputs["W1"], np.float32) @ M[1]
    W1q = W1f.astype(bf16)

    # ---- P1 ----
    p1 = build_p1()
    in_maps = [dict(xT=np.ascontiguousarray(xT[:, cols[c]]).astype(bf16), W1=W1q)
               for c in cores]
    r1 = note(run_bass_kernel_spmd(p1, in_maps, cores))
    times = [r1.exec_time_ns]

    def assemble(slabs):
        Tf = np.zeros((INP, HC), bf16)
        for c in cores:
            Tf[cols[c]] = slabs[c]
        Tf[PAD_IDX] = pad_row().astype(bf16)
        return Tf

    Tfull = assemble([r1.results[c]["Tout"] for c in cores])

    # ---- P2 (layers 2, 3) ----
    p2 = build_p23(K, with_next=True, with_head=False)
    xprev = [np.zeros((P, 1024), np.float32) for _ in cores]
    for l in (2, 3):
        adpr = np.broadcast_to(ADP[l - 1].astype(bf16), (P, HC)).copy()
        Wn = np.ascontiguousarray(np.asarray(inputs[f"W{l}"], np.float32) @ M[l])
        in_maps = []
        for c in cores:
            in_maps.append(dict(
                Tfull=Tfull, Town=np.ascontiguousarray(Tfull[cols[c]]),
                xprev=xprev[c], idxq=prep["idxq"][c],
                bprev=np.asarray(inputs[f"b{l-1}"], np.float32).reshape(P, 1),
                gam=np.asarray(inputs[f"g{l-1}"], np.float32).reshape(P, 1),
                bet=np.asarray(inputs[f"be{l-1}"], np.float32).reshape(P, 1),
                adpr=adpr, Bmat=B[l - 1].astype(bf16), rep16q=_REP16, Wn=Wn,
            ))
        r2 = note(run_bass_kernel_spmd(p2, in_maps, cores))
        times.append(r2.exec_time_ns)
        Tfull = assemble([r2.results[c]["Tout"] for c in cores])
        for c in cores:
            xprev[c] = r2.results[c]["xnout"]

    # ---- P3 (layer-3 aggregation + MLP head) ----
    p3 = build_p23(K, with_next=False, with_head=True)
    adpr3 = np.broadcast_to(ADP[3].astype(bf16), (P, HC)).copy()
    in_maps = []
    for c in cores:
        in_maps.append(dict(
            Tfull=Tfull, Town=np.ascontiguousarray(Tfull[cols[c]]),
            xprev=xprev[c], idxq=prep["idxq"][c],
            bprev=np.asarray(inputs["b3"], np.float32).reshape(P, 1),
            gam=np.asarray(inputs["g3"], np.float32).reshape(P, 1),
            bet=np.asarray(inputs["be3"], np.float32).reshape(P, 1),
            adpr=adpr3, Bmat=B[3].astype(bf16), rep16q=_REP16,
            aW1=np.ascontiguousarray(np.asarray(inputs["aW1"], np.float32)),
            ab1=np.asarray(inputs["ab1"], np.float32).reshape(P, 1),
            agm=np.asarray(inputs["ag"], np.float32).reshape(P, 1),
            abe=np.asarray(inputs["abe"], np.float32).reshape(P, 1),
            aW2=np.asarray(inputs["aW2"], np.float32).reshape(P, 1),
            ab2=np.asarray(inputs["ab2"], np.float32).reshape(1, 1),
            rW1=np.ascontiguousarray(np.asarray(inputs["rW1"], np.float32)),
            rb1=np.asarray(inputs["rb1"], np.float32).reshape(64, 1),
            rgm=np.asarray(inputs["rg"], np.float32).reshape(64, 1),
            rbe=np.asarray(inputs["rbe"], np.float32).reshape(64, 1),
            rW2=np.asarray(inputs["rW2"], np.float32).reshape(64, 1),
            rb2=np.asarray(inputs["rb2"], np.float32).reshape(1, 1),
        ))
    r3 = note(run_bass_kernel_spmd(p3, in_maps, cores))
    times.append(r3.exec_time_ns)
    ang = np.zeros(N, np.float32)
    rad = np.zeros(N, np.float32)
    for c in cores:
        ang[cols[c]] = r3.results[c]["ang"][0]
        rad[cols[c]] = r3.results[c]["rad"][0]

    # ---- P4 (finalize, replicated) ----
    p4 = build_p4()
    r4 = note(run_bass_kernel_spmd(
        p4, [dict(ANG=ang.reshape(P, 64), RAD=rad.reshape(P, 64))] * NCORES, cores))
    times.append(r4.exec_time_ns)
    cxv = r4.results[0]["CX"].reshape(N)
    cyv = r4.results[0]["CY"].reshape(N)

    out = np.zeros((N, 2), np.float32)
    out[order, 0] = cxv
    out[order, 1] = cyv
    kernel._last_times = times
    kernel._last_traces = traces
    return out


# revision 13
# speedup vs baseline: 1.5730x; 1.1133x over previous
"""Trainium2 Bass kernel for nn_GAT_86045374808682 (3-layer GAT + coordinate head).

Self-contained: takes FULL inputs, shards across 8 NeuronCores internally,
returns the FULL [8192, 2] float32 output.

v2 strategy (vs v1 fp32/768B-row baseline):
- Nodes relabeled by in-degree desc; 64 blocks of 128 striped across 8 cores
  (block j -> core j%8); per-stripe padded degree schedule K[t] (SPMD).
- Orthogonal per-head basis fold (U-trick): per head, rotate the 16-dim block
  by Q with first direction = a_src, scaled so the per-edge source score IS
  element h*16 of the row. Table row = 128 bf16 = 256 B (dma_gather minimum).
  Host folds Q*D into W (and its inverse B / a_dst' for the device).
- Gathers round-robin over 4 SWDGE queues (4x Q7 descriptor-prep parallelism),
  single_packet=False. Invalid slots gather a pad row (8192) whose score
  elements are -30000 -> exp()=0 (no mask tensor).
- Edge phase node-major: scores from strided slice, exp on ACT (only ACT table
  in P2), w = g*ex (DVE), k-sum via DVE reduce, den via reduce; per stripe:
  1/den normalize, one PE transpose, B-matmul back to h-basis, +bias.
- Batched LN over all 1024 cols: DVE square, Newton rsqrt (bit-trick seed),
  per-partition affine+relu on DVE. No ACT table swaps.
- P1: x/W1 in fp8 e4m3 (W1 pre-scaled x64, unscaled on PSUM copy-out).
- 5 launches: P1, P2 x2 (edge+node+pack), P3 (edge+head), P4 (trig finalize).
  Host concats slabs between launches (free in HW time).
"""
import sys

import numpy as np

for _p in ("/opt/trn_rl_repo", "/root/.axon_site/_ro/trn_rl_repo"):
    if _p not in sys.path:
        sys.path.append(_p)

import ml_dtypes

import concourse.bass as bass  # noqa: F401
import concourse.tile as tile
from concourse import bacc, library_config, mybir
from concourse.masks import make_identity

dt = mybir.dt
AF = mybir.ActivationFunctionType
OP = mybir.AluOpType

N = 8192
IN = 8193
INP = 8320  # 65 * 128
H = 8
HC = 128
P = 128
NCORES = 8
NSTRIPE = 8
KC = 16  # gather chunk (slots)
NQ = 4  # swdge queues
PAD_IDX = 8192
PADVAL = -30000.0
W1SCALE = 64.0
PI = float(np.pi)
MAGIC = 0x5F3759DF


# ----------------------------------------------------------------------------
# host-side prep
# ----------------------------------------------------------------------------

def host_prep(src, dst):
    s = np.concatenate([np.asarray(src).astype(np.int64), np.arange(N, dtype=np.int64)])
    d = np.concatenate([np.asarray(dst).astype(np.int64), np.arange(N, dtype=np.int64)])
    deg = np.bincount(d, minlength=N)
    order = np.argsort(-deg, kind="stable")  # new-id -> old-id
    old2new = np.empty(N, np.int64)
    old2new[order] = np.arange(N)
    s_new = old2new[s]
    d_new = old2new[d]
    deg_new = deg[order]

    K = [int(deg_new[1024 * t]) for t in range(NSTRIPE)]  # desc-sorted -> stripe max
    offs = np.cumsum([0] + K)

    eo = np.argsort(d_new, kind="stable")
    s_sorted = s_new[eo]
    starts = np.searchsorted(d_new[eo], np.arange(N))

    idxq = np.zeros((NCORES, 16, int(offs[-1]) * 8), np.int16)
    ar = np.arange(P)
    for c in range(NCORES):
        for t in range(NSTRIPE):
            Kt = K[t]
            vids = (t * NCORES + c) * P + ar
            e0 = starts[vids]
            degs = deg_new[vids]
            kk = np.arange(Kt)
            take = np.minimum(e0[:, None] + kk[None, :], len(s_sorted) - 1)
            mat = s_sorted[take]                      # [128, Kt]
            valid = kk[None, :] < degs[:, None]
            mat = np.where(valid, mat, PAD_IDX)
            lin = mat.T.reshape(-1)                   # slot-major [Kt*128]
            o16 = int(offs[t]) * 8
            idxq[c, :, o16 : o16 + Kt * 8] = lin.reshape(-1, 16).T
    return dict(order=order, K=K, offs=offs, idxq=idxq.astype(np.int16))


def core_cols(c):
    return np.concatenate([np.arange((t * NCORES + c) * P, (t * NCORES + c) * P + P)
                           for t in range(NSTRIPE)])


def fold_basis(a_src, a_dst):
    """Per-head orthogonal fold. Returns M [128,128] (fold into W: W' = W@M),
    B [128,128] (unfold: h = g@B), adp [128] (da = g . adp)."""
    M = np.zeros((HC, HC), np.float64)
    B = np.zeros((HC, HC), np.float64)
    adp = np.zeros(HC, np.float64)
    for h in range(H):
        a = np.asarray(a_src[h], np.float64)
        nrm = float(np.linalg.norm(a))
        j = int(np.argmax(np.abs(a))) if nrm > 0 else 0
        cols = [a if nrm > 0 else np.eye(16)[:, 0]]
        for i in range(16):
            if i != j:
                cols.append(np.eye(16)[:, i])
        A = np.stack(cols, axis=1)
        Q, R = np.linalg.qr(A)
        if R[0, 0] < 0:
            Q[:, 0] = -Q[:, 0]
        D = np.eye(16)
        D[0, 0] = nrm if nrm > 0 else 1.0
        Mh = Q @ D
        Bh = np.diag(1.0 / np.diag(D)) @ Q.T
        sl = slice(h * 16, (h + 1) * 16)
        M[sl, sl] = Mh
        B[sl, sl] = Bh
        adp[h * 16 : (h + 1) * 16] = Bh @ np.asarray(a_dst[h], np.float64)
    return M.astype(np.float32), B.astype(np.float32), adp.astype(np.float32)


def pad_row():
    r = np.zeros(HC, np.float32)
    for h in range(H):
        r[h * 16] = PADVAL
    return r


# ----------------------------------------------------------------------------
# device building blocks
# ----------------------------------------------------------------------------

def _newton_rsqrt(nc, sb, magic_t, v_ap, out_ap, w, tag, iters=2):
    """out = 1/sqrt(v) via magic-number seed + Newton. v > 0, fp32, [1, w]."""
    iv = sb.tile([1, 512], dt.int32, tag=f"{tag}_iv")
    nc.vector.tensor_scalar(out=iv[:, 0:w], in0=v_ap.bitcast(dt.int32), scalar1=1,
                            scalar2=None, op0=OP.logical_shift_right)
    nc.vector.tensor_tensor(out=iv[:, 0:w], in0=magic_t[:, 0:w], in1=iv[:, 0:w],
                            op=OP.subtract)
    y = iv.bitcast(dt.float32)
    u = sb.tile([1, 512], dt.float32, tag=f"{tag}_u")
    for _ in range(iters):
        nc.vector.tensor_tensor(out=u[:, 0:w], in0=y[:, 0:w], in1=y[:, 0:w], op=OP.mult)
        nc.vector.tensor_tensor(out=u[:, 0:w], in0=u[:, 0:w], in1=v_ap, op=OP.mult)
        nc.vector.tensor_scalar(out=u[:, 0:w], in0=u[:, 0:w], scalar1=-0.5,
                                scalar2=1.5, op0=OP.mult, op1=OP.add)
        nc.vector.tensor_tensor(out=y[:, 0:w], in0=y[:, 0:w], in1=u[:, 0:w], op=OP.mult)
    nc.vector.tensor_copy(out=out_ap, in_=y[:, 0:w])


def _ln_relu_fm(nc, sb, ps, c, x_sb, n, gam_t, bet_t, out_sb, nfeat=P, eps=1e-5):
    """Feature-major LN+affine+ReLU, DVE-only elementwise (no ACT tables)."""
    for j in range(0, n, 512):
        w = min(512, n - j)
        xs = x_sb[:, j : j + w]
        xsq = sb.tile([nfeat, 512], dt.float32, tag="ln_xsq")
        nc.vector.tensor_tensor(out=xsq[:, 0:w], in0=xs, in1=xs, op=OP.mult)
        s1_ps = ps.tile([1, 512], dt.float32, space="PSUM", tag="pp_a")
        nc.tensor.matmul(out=s1_ps[:, 0:w], lhsT=c["ones_col"][0:nfeat, :], rhs=xs,
                         start=True, stop=True)
        s2_ps = ps.tile([1, 512], dt.float32, space="PSUM", tag="pp_b")
        nc.tensor.matmul(out=s2_ps[:, 0:w], lhsT=c["ones_col"][0:nfeat, :],
                         rhs=xsq[:, 0:w], start=True, stop=True)
        mu = sb.tile([1, 512], dt.float32, tag="ln_mu")
        nc.vector.tensor_scalar_mul(mu[:, 0:w], s1_ps[:, 0:w], 1.0 / nfeat)
        musq = sb.tile([1, 512], dt.float32, tag="ln_musq")
        nc.vector.tensor_tensor(out=musq[:, 0:w], in0=mu[:, 0:w], in1=mu[:, 0:w],
                                op=OP.mult)
        var = sb.tile([1, 512], dt.float32, tag="ln_var")
        nc.vector.scalar_tensor_tensor(out=var[:, 0:w], in0=s2_ps[:, 0:w],
                                       scalar=1.0 / nfeat, in1=musq[:, 0:w],
                                       op0=OP.mult, op1=OP.subtract)
        nc.vector.tensor_scalar_add(var[:, 0:w], var[:, 0:w], float(eps))
        rs = sb.tile([1, 512], dt.float32, tag="ln_rs")
        _newton_rsqrt(nc, sb, c["magic"], var[:, 0:w], rs[:, 0:w], w, "ln")
        rep_mu = ps.tile([nfeat, 512], dt.float32, space="PSUM", tag="pp_a")
        nc.tensor.matmul(out=rep_mu[:, 0:w], lhsT=c["ones_row"][:, 0:nfeat],
                         rhs=mu[:, 0:w], start=True, stop=True)
        rep_rs = ps.tile([nfeat, 512], dt.float32, space="PSUM", tag="pp_b")
        nc.tensor.matmul(out=rep_rs[:, 0:w], lhsT=c["ones_row"][:, 0:nfeat],
                         rhs=rs[:, 0:w], start=True, stop=True)
        xh = sb.tile([nfeat, 512], dt.float32, tag="ln_xh")
        nc.vector.tensor_tensor(out=xh[:, 0:w], in0=xs, in1=rep_mu[:, 0:w],
                                op=OP.subtract)
        nc.vector.tensor_tensor(out=xh[:, 0:w], in0=xh[:, 0:w], in1=rep_rs[:, 0:w],
                                op=OP.mult)
        nc.vector.tensor_scalar(out=xh[:, 0:w], in0=xh[:, 0:w], scalar1=gam_t[:],
                                scalar2=bet_t[:], op0=OP.mult, op1=OP.add)
        nc.vector.tensor_scalar_max(out_sb[:, j : j + w], xh[:, 0:w], 0.0)


def _mk_consts(nc, consts, need_gather=True):
    c = {}
    if need_gather:
        nc.gpsimd.load_library(library_config.mlp)
    c["ident"] = consts.tile([P, P], dt.float16, name="c_ident")
    make_identity(nc, c["ident"][:])
    c["ones_col"] = consts.tile([P, 1], dt.float32, name="c_ones_col")
    nc.gpsimd.memset(c["ones_col"][:], 1.0)
    c["ones_row"] = consts.tile([1, P], dt.float32, name="c_ones_row")
    nc.gpsimd.memset(c["ones_row"][:], 1.0)
    c["magic"] = consts.tile([1, 512], dt.int32, name="c_magic")
    nc.gpsimd.memset(c["magic"][:], MAGIC)
    return c


# ----------------------------------------------------------------------------
# P1: g1 = (x @ W1') in fp8, pack bf16 rows
# ----------------------------------------------------------------------------

def build_p1():
    nc = bacc.Bacc(None, target_bir_lowering=False)
    xT = nc.declare_dram_parameter("xT", [INP, 1024], dt.float16, isOutput=False)
    W1 = nc.declare_dram_parameter("W1", [INP, HC], dt.float16, isOutput=False)
    Tout = nc.declare_dram_parameter("Tout", [1024, HC], dt.float16, isOutput=True)

    with tile.TileContext(nc) as tc:
        with (
            tc.tile_pool(name="consts", bufs=1) as consts,
            tc.tile_pool(name="xp", bufs=3) as xp,
            tc.tile_pool(name="sb", bufs=2) as sb,
            tc.tile_pool(name="psh", bufs=2, space="PSUM") as psh,
            tc.tile_pool(name="ps", bufs=2, space="PSUM") as ps,
        ):
            ident = consts.tile([P, P], dt.float16)
            make_identity(nc, ident[:])
            w1_t = consts.tile([P, 65, P], dt.float16)
            nc.sync.dma_start(out=w1_t[:], in_=W1.rearrange("(k p) f -> p k f", p=P))
            xTr = xT.rearrange("(k p) n -> p k n", p=P)
            GB = 4
            hps0 = psh.tile([P, 512], dt.float32, space="PSUM", tag="hps0")
            hps1 = psh.tile([P, 512], dt.float32, space="PSUM", tag="hps1")
            ngrp = (65 + GB - 1) // GB
            for gi in range(ngrp):
                k0 = gi * GB
                nk = min(GB, 65 - k0)
                xt = xp.tile([P, GB, 1024], dt.float16, tag="xt")
                eng = nc.sync if gi % 2 == 0 else nc.scalar
                eng.dma_start(out=xt[:, 0:nk, :], in_=xTr[:, k0 : k0 + nk, :])
                for kk in range(nk):
                    kcb = k0 + kk
                    nc.tensor.matmul(out=hps0[:], lhsT=w1_t[:, kcb, :],
                                     rhs=xt[:, kk, 0:512],
                                     start=(kcb == 0), stop=(kcb == 64))
                    nc.tensor.matmul(out=hps1[:], lhsT=w1_t[:, kcb, :],
                                     rhs=xt[:, kk, 512:1024],
                                     start=(kcb == 0), stop=(kcb == 64))
            for nb in range(2):
                hps = hps0 if nb == 0 else hps1
                h_sb = sb.tile([P, 512], dt.float16, tag="h_sb")
                nc.vector.tensor_copy(out=h_sb[:], in_=hps[:])
                for b in range(4):
                    blk = nb * 4 + b
                    ht_ps = ps.tile([P, P], dt.float16, space="PSUM", tag="pp_t")
                    nc.tensor.matmul(out=ht_ps[:], lhsT=h_sb[:, b * P : (b + 1) * P],
                                     rhs=ident[:], is_transpose=True, start=True,
                                     stop=True)
                    pk = sb.tile([P, P], dt.float16, tag="pk")
                    nc.vector.tensor_copy(out=pk[:], in_=ht_ps[:])
                    nc.sync.dma_start(out=Tout[blk * P : (blk + 1) * P, :], in_=pk[:])
    nc.finalize()
    return nc


# ----------------------------------------------------------------------------
# P2/P3: edge aggregation + node phase (+ pack next | + MLP head)
# ----------------------------------------------------------------------------

def build_p23(K, with_next, with_head):
    SK = int(sum(K))
    offs = np.cumsum([0] + list(K))
    nc = bacc.Bacc(None, target_bir_lowering=False, num_swdge_queues=NQ,
                   num_devices=NCORES)
    Tfull = nc.declare_dram_parameter("Tfull", [INP, HC], dt.float16, isOutput=False)
    Town = nc.declare_dram_parameter("Town", [1024, HC], dt.float16, isOutput=False)
    xprev = nc.declare_dram_parameter("xprev", [P, 1024], dt.float32, isOutput=False)
    idxq = nc.declare_dram_parameter("idxq", [16, SK * 8], dt.int16, isOutput=False)
    bprev = nc.declare_dram_parameter("bprev", [P, 1], dt.float32, isOutput=False)
    gam = nc.declare_dram_parameter("gam", [P, 1], dt.float32, isOutput=False)
    bet = nc.declare_dram_parameter("bet", [P, 1], dt.float32, isOutput=False)
    adpr = nc.declare_dram_parameter("adpr", [P, HC], dt.float16, isOutput=False)
    Bmat = nc.declare_dram_parameter("Bmat", [P, P], dt.float32, isOutput=False)
    rep16q = nc.declare_dram_parameter("rep16q", [H, P], dt.float32, isOutput=False)
    if with_next:
        Wn = nc.declare_dram_parameter("Wn", [P, P], dt.float32, isOutput=False)
        Tout = nc.declare_dram_parameter("Tout", [1024, HC], dt.float16, isOutput=True)
        xnout = nc.declare_dram_parameter("xnout", [P, 1024], dt.float32, isOutput=True)
    if with_head:
        aW1 = nc.declare_dram_parameter("aW1", [P, P], dt.float32, isOutput=False)
        ab1 = nc.declare_dram_parameter("ab1", [P, 1], dt.float32, isOutput=False)
        agm = nc.declare_dram_parameter("agm", [P, 1], dt.float32, isOutput=False)
        abe = nc.declare_dram_parameter("abe", [P, 1], dt.float32, isOutput=False)
        aW2 = nc.declare_dram_parameter("aW2", [P, 1], dt.float32, isOutput=False)
        ab2 = nc.declare_dram_parameter("ab2", [1, 1], dt.float32, isOutput=False)
        rW1 = nc.declare_dram_parameter("rW1", [P, 64], dt.float32, isOutput=False)
        rb1 = nc.declare_dram_parameter("rb1", [64, 1], dt.float32, isOutput=False)
        rgm = nc.declare_dram_parameter("rgm", [64, 1], dt.float32, isOutput=False)
        rbe = nc.declare_dram_parameter("rbe", [64, 1], dt.float32, isOutput=False)
        rW2 = nc.declare_dram_parameter("rW2", [64, 1], dt.float32, isOutput=False)
        rb2 = nc.declare_dram_parameter("rb2", [1, 1], dt.float32, isOutput=False)
        COX = nc.declare_dram_parameter("COX", [P, 8], dt.float32, isOutput=True)
        COY = nc.declare_dram_parameter("COY", [P, 8], dt.float32, isOutput=True)
        Scc = nc.dram_tensor("Scc", (1, 2), dt.float32, kind="Internal")
        Rcc = nc.dram_tensor("Rcc", (1, 2), dt.float32, kind="Internal")
        AngD = nc.dram_tensor("AngD", (1, 1024), dt.float32, kind="Internal")
        RadD = nc.dram_tensor("RadD", (1, 1024), dt.float32, kind="Internal")

    with tile.TileContext(nc) as tc:
        with (
            tc.tile_pool(name="consts", bufs=1) as consts,
            tc.tile_pool(name="gpool", bufs=8) as gpool,
            tc.tile_pool(name="wpool", bufs=3) as wpool,
            tc.tile_pool(name="sc", bufs=3) as sc,
            tc.tile_pool(name="sb", bufs=1) as sb,
            tc.tile_pool(name="ps", bufs=2, space="PSUM") as ps,
            tc.tile_pool(name="pst", bufs=1, space="PSUM") as pst,
            tc.tile_pool(name="psagg", bufs=2, space="PSUM") as psagg,
        ):
            c = _mk_consts(nc, consts)
            c["identf"] = consts.tile([P, P], dt.float32, name="c_identf")
            make_identity(nc, c["identf"][:])
            rep16_t = consts.tile([H, P], dt.float32, name="c_rep16")
            nc.sync.dma_start(out=rep16_t[:], in_=rep16q[:])

            idx_t = sb.tile([P, SK * 8], dt.int16, tag="idx")
            for a in range(8):
                nc.sync.dma_start(out=idx_t[16 * a : 16 * (a + 1), :], in_=idxq[:])
            town_t = sb.tile([P, NSTRIPE, HC], dt.float16, tag="town")
            nc.sync.dma_start(out=town_t[:],
                              in_=Town.rearrange("(t p) f -> p t f", p=P))
            xprev_t = sb.tile([P, 1024], dt.float32, tag="xprev")
            nc.sync.dma_start(out=xprev_t[:], in_=xprev[:])
            adpr_t = sb.tile([P, HC], dt.float16, tag="adpr")
            nc.sync.dma_start(out=adpr_t[:], in_=adpr[:])
            bias_t = sb.tile([P, 1], dt.float32, tag="bias")
            nc.sync.dma_start(out=bias_t[:], in_=bprev[:])
            gam_t = sb.tile([P, 1], dt.float32, tag="gam")
            nc.sync.dma_start(out=gam_t[:], in_=gam[:])
            bet_t = sb.tile([P, 1], dt.float32, tag="bet")
            nc.sync.dma_start(out=bet_t[:], in_=bet[:])
            bmat_t = sb.tile([P, P], dt.float32, tag="bmat")
            nc.sync.dma_start(out=bmat_t[:], in_=Bmat[:])
            if with_next:
                wn_t = sb.tile([P, P], dt.float32, tag="wn")
                nc.sync.dma_start(out=wn_t[:], in_=Wn[:])

            # da[p, t, h] = sum_j town[p, t, h*16+j] * adp[h*16+j]
            dam = sb.tile([P, NSTRIPE, HC], dt.float16, tag="dam")
            nc.vector.tensor_tensor(
                out=dam[:], in0=town_t[:],
                in1=adpr_t.unsqueeze(1).to_broadcast([P, NSTRIPE, HC]), op=OP.mult)
            da_t = sb.tile([P, NSTRIPE, H], dt.float32, tag="da")
            nc.vector.tensor_reduce(
                out=da_t[:], in_=dam[:].rearrange("p t (h j) -> p t h j", j=16),
                axis=mybir.AxisListType.X, op=OP.add)

            xball = sb.tile([P, 1024], dt.float32, tag="xball")
            gq = [0]

            for t in range(NSTRIPE):
                K_t, off_t = K[t], int(offs[t])
                nchunk = (K_t + KC - 1) // KC
                agg_ps = psagg.tile([P, P], dt.float32, space="PSUM", tag="agg")
                den = sc.tile([P, H], dt.float32, tag="den")
                for ci in range(nchunk):
                    k0 = ci * KC
                    kc = min(KC, K_t - k0)
                    g = gpool.tile([P, KC, HC], dt.float16, tag="g")
                    nc.gpsimd.dma_gather(
                        out_ap=g[:, 0:kc, :],
                        in_ap=Tfull[:],
                        idxs_ap=idx_t[:, (off_t + k0) * 8 : (off_t + k0 + kc) * 8],
                        num_idxs=kc * P,
                        num_idxs_reg=kc * P,
                        elem_size=HC,
                        single_packet=False,
                        queue_num=gq[0] % NQ,
                    )
                    gq[0] += 1
                    g4 = g[:, 0:kc, :].rearrange("p k (h j) -> p k h j", j=16)
                    z = sc.tile([P, KC, H], dt.float32, tag="z")
                    nc.vector.tensor_tensor(
                        out=z[:, 0:kc, :],
                        in0=g4[:, :, :, 0:1].rearrange("p k h j -> p k (h j)"),
                        in1=da_t[:, t, :].unsqueeze(1).to_broadcast([P, kc, H]),
                        op=OP.add)
                    zl = sc.tile([P, KC, H], dt.float32, tag="zl")
                    nc.vector.scalar_tensor_tensor(out=zl[:, 0:kc, :], in0=z[:, 0:kc, :],
                                                   scalar=0.2, in1=z[:, 0:kc, :],
                                                   op0=OP.mult, op1=OP.max)
                    ex8 = sc.tile([P, KC, H], dt.float16, tag="ex8")
                    nc.scalar.activation(out=ex8[:, 0:kc, :], in_=zl[:, 0:kc, :],
                                         func=AF.Exp)
                    dc = sc.tile([P, H], dt.float32, tag="dc")
                    nc.vector.tensor_reduce(out=dc[:],
                                            in_=ex8[:, 0:kc, :].transpose([0, 2, 1]),
                                            axis=mybir.AxisListType.X, op=OP.add)
                    if ci == 0:
                        nc.vector.tensor_copy(out=den[:], in_=dc[:])
                    else:
                        nc.vector.tensor_tensor(out=den[:], in0=den[:], in1=dc[:],
                                                op=OP.add)
                    exr = wpool.tile([P, KC, HC], dt.float16, tag="exr")
                    nc.scalar.activation(
                        out=exr[:, 0:kc, :].rearrange("p k (h j) -> p k h j", j=16),
                        in_=zl[:, 0:kc, :].unsqueeze(3).to_broadcast([P, kc, H, 16]),
                        func=AF.Exp)
                    w = wpool.tile([P, KC, HC], dt.float16, tag="w")
                    nc.vector.tensor_tensor(
                        out=w[:, 0:kc, :].rearrange("p k f -> p (k f)"),
                        in0=g[:, 0:kc, :].rearrange("p k f -> p (k f)"),
                        in1=exr[:, 0:kc, :].rearrange("p k f -> p (k f)"),
                        op=OP.mult)
                    hk = kc // 2
                    nslot = hk + (kc - 2 * hk)
                    w8 = wpool.tile([P, KC // 2 + 1, HC], dt.float16, tag="w8")
                    if hk:
                        nc.vector.tensor_tensor(out=w8[:, 0:hk, :], in0=w[:, 0:hk, :],
                                                in1=w[:, hk : 2 * hk, :], op=OP.add)
                    if kc - 2 * hk:
                        nc.vector.tensor_copy(out=w8[:, hk, :], in_=w[:, kc - 1, :])
                    for k in range(nslot):
                        nc.tensor.matmul(out=agg_ps[:], lhsT=w8[:, k, :],
                                         rhs=c["ident"][:],
                                         start=(ci == 0 and k == 0),
                                         stop=(ci == nchunk - 1 and k == nslot - 1))
                dent = pst.tile([H, P], dt.float32, space="PSUM", tag="pp_d")
                nc.tensor.matmul(out=dent[0:H, :], lhsT=den[:], rhs=c["identf"][:],
                                 start=True, stop=True)
                rden = sc.tile([H, P], dt.float32, tag="rden")
                nc.vector.reciprocal(out=rden[:], in_=dent[0:H, :])
                rdrep = pst.tile([P, P], dt.float32, space="PSUM", tag="pp_d")
                nc.tensor.matmul(out=rdrep[:], lhsT=rep16_t[:], rhs=rden[:],
                                 start=True, stop=True)
                rdsb = sc.tile([P, P], dt.float32, tag="rdsb")
                nc.vector.tensor_copy(out=rdsb[:], in_=rdrep[:])
                aggn = sc.tile([P, P], dt.float32, tag="aggn")
                nc.vector.tensor_tensor(out=aggn[:], in0=agg_ps[:], in1=rdsb[:],
                                        op=OP.mult)
                hps = pst.tile([P, P], dt.float32, space="PSUM", tag="pp_h")
                nc.tensor.matmul(out=hps[:], lhsT=bmat_t[:], rhs=aggn[:],
                                 start=True, stop=True)
                nc.vector.tensor_scalar_add(xball[:, t * P : (t + 1) * P], hps[:],
                                            bias_t[:])

            # node phase, batched over all 1024 columns
            xo = sb.tile([P, 1024], dt.float32, tag="xo")
            _ln_relu_fm(nc, sb, ps, c, xball[:], 1024, gam_t, bet_t, xo[:])
            xnext = sb.tile([P, 1024], dt.float32, tag="xnext")
            nc.vector.tensor_tensor(out=xnext[:], in0=xo[:], in1=xprev_t[:], op=OP.add)

            if with_next:
                for j in range(0, 1024, 512):
                    gps = ps.tile([P, 512], dt.float32, space="PSUM", tag="pp_a")
                    nc.tensor.matmul(out=gps[:], lhsT=wn_t[:],
                                     rhs=xnext[:, j : j + 512], start=True, stop=True)
                    gsb = sb.tile([P, 512], dt.float16, tag="gsb")
                    nc.vector.tensor_copy(out=gsb[:], in_=gps[:])
                    for b in range(4):
                        blk = j // P + b
                        tp2 = pst.tile([P, P], dt.float16, space="PSUM", tag="pp_h")
                        nc.tensor.matmul(out=tp2[:], lhsT=gsb[:, b * P : (b + 1) * P],
                                         rhs=c["ident"][:], is_transpose=True,
                                         start=True, stop=True)
                        pk = sb.tile([P, P], dt.float16, tag="pk")
                        nc.vector.tensor_copy(out=pk[:], in_=tp2[:])
                        nc.sync.dma_start(out=Tout[blk * P : (blk + 1) * P, :],
                                          in_=pk[:])
                nc.sync.dma_start(out=xnout[:], in_=xnext[:])

            if with_head:
                n = 1024
                # h3n = xnext / row_norm
                xsq = sb.tile([P, n], dt.float32, tag="hd_xsq")
                nc.vector.tensor_tensor(out=xsq[:], in0=xnext[:], in1=xnext[:],
                                        op=OP.mult)
                h3n = sb.tile([P, n], dt.float32, tag="hd_h3n")
                for j in range(0, n, 512):
                    ss_ps = ps.tile([1, 512], dt.float32, space="PSUM", tag="pp_a")
                    nc.tensor.matmul(out=ss_ps[0:1, :], lhsT=c["ones_col"][:],
                                     rhs=xsq[:, j : j + 512], start=True, stop=True)
                    ss = sb.tile([1, 512], dt.float32, tag="hd_ss")
                    nc.vector.tensor_scalar_max(ss[:], ss_ps[0:1, :], 1e-24)
                    rn = sb.tile([1, 512], dt.float32, tag="hd_rn")
                    _newton_rsqrt(nc, sb, c["magic"], ss[:], rn[:], 512, "hd", iters=3)
                    rn_rep = ps.tile([P, 512], dt.float32, space="PSUM", tag="pp_b")
                    nc.tensor.matmul(out=rn_rep[:], lhsT=c["ones_row"][:], rhs=rn[:],
                                     start=True, stop=True)
                    nc.vector.tensor_tensor(out=h3n[:, j : j + 512],
                                            in0=xnext[:, j : j + 512],
                                            in1=rn_rep[:], op=OP.mult)

                def mm_bias(lhsT_t, rhs_sb, m, bias_ap, out_sb):
                    for j in range(0, n, 512):
                        mm_ps = ps.tile([P, 512], dt.float32, space="PSUM", tag="pp_a")
                        nc.tensor.matmul(out=mm_ps[0:m, :], lhsT=lhsT_t,
                                         rhs=rhs_sb[:, j : j + 512], start=True,
                                         stop=True)
                        nc.vector.tensor_scalar_add(out_sb[0:m, j : j + 512],
                                                    mm_ps[0:m, :], bias_ap)

                aW1_t = sb.tile([P, P], dt.float32, tag="hd_aW1")
                nc.sync.dma_start(out=aW1_t[:], in_=aW1[:])
                ab1_t = sb.tile([P, 1], dt.float32, tag="hd_ab1")
                nc.sync.dma_start(out=ab1_t[:], in_=ab1[:])
                agm_t = sb.tile([P, 1], dt.float32, tag="hd_agm")
                nc.sync.dma_start(out=agm_t[:], in_=agm[:])
                abe_t = sb.tile([P, 1], dt.float32, tag="hd_abe")
                nc.sync.dma_start(out=abe_t[:], in_=abe[:])
                a_pre = sb.tile([P, n], dt.float32, tag="hd_apre")
                mm_bias(aW1_t[:], h3n, P, ab1_t[:], a_pre)
                a_hid = sb.tile([P, n], dt.float32, tag="hd_ahid")
                _ln_relu_fm(nc, sb, ps, c, a_pre[:], n, agm_t, abe_t, a_hid[:])

                aW2_t = sb.tile([P, 1], dt.float32, tag="hd_aW2")
                nc.sync.dma_start(out=aW2_t[:], in_=aW2[:])
                ab2_t = sb.tile([1, 1], dt.float32, tag="hd_ab2")
                nc.sync.dma_start(out=ab2_t[:], in_=ab2[:])
                av = sb.tile([1, n], dt.float32, tag="hd_av")
                mm_bias(aW2_t[:], a_hid, 1, ab2_t[:], av)
                # angles = pi*tanh(av) = pi - 2pi/(exp(2av)+1)
                e2 = sb.tile([1, n], dt.float32, tag="hd_e2")
                nc.scalar.activation(out=e2[:], in_=av[:], func=AF.Exp, scale=2.0)
                nc.vector.tensor_scalar_add(e2[:], e2[:], 1.0)
                rr = sb.tile([1, n], dt.float32, tag="hd_rr")
                nc.vector.reciprocal(out=rr[:], in_=e2[:])
                angv = sb.tile([1, n], dt.float32, tag="hd_angv")
                nc.vector.tensor_scalar(out=angv[:], in0=rr[:], scalar1=-2.0 * PI,
                                        scalar2=PI, op0=OP.mult, op1=OP.add)

                rW1_t = sb.tile([P, 64], dt.float32, tag="hd_rW1")
                nc.sync.dma_start(out=rW1_t[:], in_=rW1[:])
                rb1_t = sb.tile([64, 1], dt.float32, tag="hd_rb1")
                nc.sync.dma_start(out=rb1_t[:], in_=rb1[:])
                rgm_t = sb.tile([64, 1], dt.float32, tag="hd_rgm")
                nc.sync.dma_start(out=rgm_t[:], in_=rgm[:])
                rbe_t = sb.tile([64, 1], dt.float32, tag="hd_rbe")
                nc.sync.dma_start(out=rbe_t[:], in_=rbe[:])
                r_pre = sb.tile([64, n], dt.float32, tag="hd_rpre")
                mm_bias(rW1_t[:], h3n, 64, rb1_t[:], r_pre)
                r_hid = sb.tile([64, n], dt.float32, tag="hd_rhid")
                _ln_relu_fm(nc, sb, ps, c, r_pre[:], n, rgm_t, rbe_t, r_hid[:],
                            nfeat=64)

                rW2_t = sb.tile([64, 1], dt.float32, tag="hd_rW2")
                nc.sync.dma_start(out=rW2_t[:], in_=rW2[:])
                rb2_t = sb.tile([1, 1], dt.float32, tag="hd_rb2")
                nc.sync.dma_start(out=rb2_t[:], in_=rb2[:])
                rv = sb.tile([1, n], dt.float32, tag="hd_rv")
                for j in range(0, n, 512):
                    mm_ps = ps.tile([1, 512], dt.float32, space="PSUM", tag="pp_a")
                    nc.tensor.matmul(out=mm_ps[0:1, :], lhsT=rW2_t[:],
                                     rhs=r_hid[:, j : j + 512], start=True, stop=True)
                    nc.vector.tensor_scalar_add(rv[:, j : j + 512], mm_ps[0:1, :],
                                                rb2_t[:])
                # softplus then radius = 1 + 0.1 tanh(sp) = 1.1 - 0.2/(exp(2 sp)+1)
                sp = sb.tile([1, n], dt.float32, tag="hd_sp")
                nc.scalar.activation(out=sp[:], in_=rv[:], func=AF.Exp)
                nc.vector.tensor_scalar_add(sp[:], sp[:], 1.0)
                nc.scalar.activation(out=sp[:], in_=sp[:], func=AF.Ln)
                e2r = sb.tile([1, n], dt.float32, tag="hd_e2r")
                nc.scalar.activation(out=e2r[:], in_=sp[:], func=AF.Exp, scale=2.0)
                nc.vector.tensor_scalar_add(e2r[:], e2r[:], 1.0)
                rr2 = sb.tile([1, n], dt.float32, tag="hd_rr2")
                nc.vector.reciprocal(out=rr2[:], in_=e2r[:])
                radv = sb.tile([1, n], dt.float32, tag="hd_radv")
                nc.vector.tensor_scalar(out=radv[:], in0=rr2[:], scalar1=-0.2,
                                        scalar2=1.1, op0=OP.mult, op1=OP.add)

                # ---- fold of the old P4: trig, global mean via AllReduce, norm
                half_pi = consts.tile([P, 1], dt.float32, name="c_halfpi")
                nc.gpsimd.memset(half_pi[:], PI / 2.0)
                magicp = consts.tile([P, 8], dt.int32, name="c_magicp")
                nc.gpsimd.memset(magicp[:], MAGIC)
                nc.sync.dma_start(out=AngD[:], in_=angv[:])
                nc.sync.dma_start(out=RadD[:], in_=radv[:])
                angp = sb.tile([P, 8], dt.float32, tag="f_angp")
                nc.sync.dma_start(out=angp[:],
                                  in_=AngD.rearrange("one (p j) -> (one p) j", p=P))
                radp = sb.tile([P, 8], dt.float32, tag="f_radp")
                nc.sync.dma_start(out=radp[:],
                                  in_=RadD.rearrange("one (p j) -> (one p) j", p=P))
                absang = sb.tile([P, 8], dt.float32, tag="f_abs")
                nc.vector.scalar_tensor_tensor(out=absang[:], in0=angp[:], scalar=-1.0,
                                               in1=angp[:], op0=OP.mult, op1=OP.max)
                cosx = sb.tile([P, 8], dt.float32, tag="f_cos")
                nc.scalar.activation(out=cosx[:], in_=absang[:], func=AF.Sin,
                                     scale=-1.0, bias=half_pi[:])
                sinx = sb.tile([P, 8], dt.float32, tag="f_sin")
                nc.scalar.activation(out=sinx[:], in_=angp[:], func=AF.Sin)
                cx = sb.tile([P, 8], dt.float32, tag="f_cx")
                nc.vector.tensor_tensor(out=cx[:], in0=radp[:], in1=cosx[:], op=OP.mult)
                cy = sb.tile([P, 8], dt.float32, tag="f_cy")
                nc.vector.tensor_tensor(out=cy[:], in0=radp[:], in1=sinx[:], op=OP.mult)
                colsum = sb.tile([P, 2], dt.float32, tag="f_colsum")
                nc.vector.tensor_reduce(out=colsum[:, 0:1], in_=cx[:],
                                        axis=mybir.AxisListType.X, op=OP.add)
                nc.vector.tensor_reduce(out=colsum[:, 1:2], in_=cy[:],
                                        axis=mybir.AxisListType.X, op=OP.add)
                tot_ps = pst.tile([1, 2], dt.float32, space="PSUM", tag="pp_d")
                nc.tensor.matmul(out=tot_ps[0:1, :], lhsT=c["ones_col"][:],
                                 rhs=colsum[:], start=True, stop=True)
                tot_sb = sb.tile([1, 2], dt.float32, tag="f_tot")
                nc.vector.tensor_copy(out=tot_sb[:], in_=tot_ps[0:1, :])
                nc.sync.dma_start(out=Scc[:], in_=tot_sb[:])
                nc.gpsimd.collective_compute(
                    "AllReduce", OP.add,
                    replica_groups=[list(range(NCORES))],
                    ins=[Scc[:]], outs=[Rcc[:]])
                rsum = sb.tile([1, 2], dt.float32, tag="f_rsum")
                nc.sync.dma_start(out=rsum[:], in_=Rcc[:])
                mean = sb.tile([1, 2], dt.float32, tag="f_mean")
                nc.vector.tensor_scalar_mul(mean[:], rsum[:], 1.0 / N)
                mean_rep = pst.tile([P, 2], dt.float32, space="PSUM", tag="pp_d")
                nc.tensor.matmul(out=mean_rep[:], lhsT=c["ones_row"][:], rhs=mean[:],
                                 start=True, stop=True)
                mrep_sb = sb.tile([P, 2], dt.float32, tag="f_mrep")
                nc.vector.tensor_copy(out=mrep_sb[:], in_=mean_rep[:])
                nc.vector.tensor_tensor(out=cx[:], in0=cx[:],
                                        in1=mrep_sb[:, 0:1].to_broadcast([P, 8]),
                                        op=OP.subtract)
                nc.vector.tensor_tensor(out=cy[:], in0=cy[:],
                                        in1=mrep_sb[:, 1:2].to_broadcast([P, 8]),
                                        op=OP.subtract)
                qn = sb.tile([P, 8], dt.float32, tag="f_q")
                nc.vector.tensor_tensor(out=qn[:], in0=cx[:], in1=cx[:], op=OP.mult)
                cy2 = sb.tile([P, 8], dt.float32, tag="f_cy2")
                nc.vector.tensor_tensor(out=cy2[:], in0=cy[:], in1=cy[:], op=OP.mult)
                nc.vector.tensor_tensor(out=qn[:], in0=qn[:], in1=cy2[:], op=OP.add)
                nc.vector.tensor_scalar_max(qn[:], qn[:], 1e-24)
                ivf = sb.tile([P, 8], dt.int32, tag="f_iv")
                nc.vector.tensor_scalar(out=ivf[:], in0=qn[:].bitcast(dt.int32),
                                        scalar1=1, scalar2=None,
                                        op0=OP.logical_shift_right)
                nc.vector.tensor_tensor(out=ivf[:], in0=magicp[:], in1=ivf[:],
                                        op=OP.subtract)
                yf = ivf.bitcast(dt.float32)
                uf = sb.tile([P, 8], dt.float32, tag="f_u")
                for _ in range(3):
                    nc.vector.tensor_tensor(out=uf[:], in0=yf[:], in1=yf[:], op=OP.mult)
                    nc.vector.tensor_tensor(out=uf[:], in0=uf[:], in1=qn[:], op=OP.mult)
                    nc.vector.tensor_scalar(out=uf[:], in0=uf[:], scalar1=-0.5,
                                            scalar2=1.5, op0=OP.mult, op1=OP.add)
                    nc.vector.tensor_tensor(out=yf[:], in0=yf[:], in1=uf[:], op=OP.mult)
                nc.vector.tensor_tensor(out=cx[:], in0=cx[:], in1=yf[:], op=OP.mult)
                nc.vector.tensor_tensor(out=cy[:], in0=cy[:], in1=yf[:], op=OP.mult)
                nc.sync.dma_start(out=COX[:], in_=cx[:])
                nc.sync.dma_start(out=COY[:], in_=cy[:])
    nc.finalize()
    return nc


# ----------------------------------------------------------------------------
# P4: trig finalize (replicated)
# ----------------------------------------------------------------------------

def build_p4():
    nc = bacc.Bacc(None, target_bir_lowering=False)
    ANG = nc.declare_dram_parameter("ANG", [P, 64], dt.float32, isOutput=False)
    RAD = nc.declare_dram_parameter("RAD", [P, 64], dt.float32, isOutput=False)
    CX = nc.declare_dram_parameter("CX", [P, 64], dt.float32, isOutput=True)
    CY = nc.declare_dram_parameter("CY", [P, 64], dt.float32, isOutput=True)
    with tile.TileContext(nc) as tc:
        with (
            tc.tile_pool(name="consts", bufs=1) as consts,
            tc.tile_pool(name="sb", bufs=1) as sb,
            tc.tile_pool(name="ps", bufs=1, space="PSUM") as ps,
        ):
            ones_col = consts.tile([P, 1], dt.float32)
            nc.gpsimd.memset(ones_col[:], 1.0)
            ones_row = consts.tile([1, P], dt.float32)
            nc.gpsimd.memset(ones_row[:], 1.0)
            half_pi = consts.tile([P, 1], dt.float32)
            nc.gpsimd.memset(half_pi[:], PI / 2.0)
            magic = consts.tile([P, 64], dt.int32)
            nc.gpsimd.memset(magic[:], MAGIC)

            ang_t = sb.tile([P, 64], dt.float32)
            nc.sync.dma_start(out=ang_t[:], in_=ANG[:])
            rad_t = sb.tile([P, 64], dt.float32)
            nc.sync.dma_start(out=rad_t[:], in_=RAD[:])
            absang = sb.tile([P, 64], dt.float32)
            nc.vector.scalar_tensor_tensor(out=absang[:], in0=ang_t[:], scalar=-1.0,
                                           in1=ang_t[:], op0=OP.mult, op1=OP.max)
            cosx = sb.tile([P, 64], dt.float32)
            nc.scalar.activation(out=cosx[:], in_=absang[:], func=AF.Sin,
                                 scale=-1.0, bias=half_pi[:])
            sinx = sb.tile([P, 64], dt.float32)
            nc.scalar.activation(out=sinx[:], in_=ang_t[:], func=AF.Sin)
            cx = sb.tile([P, 64], dt.float32)
            nc.vector.tensor_tensor(out=cx[:], in0=rad_t[:], in1=cosx[:], op=OP.mult)
            cy = sb.tile([P, 64], dt.float32)
            nc.vector.tensor_tensor(out=cy[:], in0=rad_t[:], in1=sinx[:], op=OP.mult)
            colsum = sb.tile([P, 2], dt.float32)
            nc.vector.tensor_reduce(out=colsum[:, 0:1], in_=cx[:],
                                    axis=mybir.AxisListType.X, op=OP.add)
            nc.vector.tensor_reduce(out=colsum[:, 1:2], in_=cy[:],
                                    axis=mybir.AxisListType.X, op=OP.add)
            tot_ps = ps.tile([1, 2], dt.float32, space="PSUM")
            nc.tensor.matmul(out=tot_ps[0:1, :], lhsT=ones_col[:], rhs=colsum[:],
                             start=True, stop=True)
            mean = sb.tile([1, 2], dt.float32)
            nc.vector.tensor_scalar_mul(mean[:], tot_ps[0:1, :], 1.0 / N)
            mean_rep = ps.tile([P, 2], dt.float32, space="PSUM")
            nc.tensor.matmul(out=mean_rep[:], lhsT=ones_row[:], rhs=mean[:],
                             start=True, stop=True)
            mrep_sb = sb.tile([P, 2], dt.float32)
            nc.vector.tensor_copy(out=mrep_sb[:], in_=mean_rep[:])
            nc.vector.tensor_tensor(out=cx[:], in0=cx[:],
                                    in1=mrep_sb[:, 0:1].to_broadcast([P, 64]),
                                    op=OP.subtract)
            nc.vector.tensor_tensor(out=cy[:], in0=cy[:],
                                    in1=mrep_sb[:, 1:2].to_broadcast([P, 64]),
                                    op=OP.subtract)
            q = sb.tile([P, 64], dt.float32)
            nc.vector.tensor_tensor(out=q[:], in0=cx[:], in1=cx[:], op=OP.mult)
            cy2 = sb.tile([P, 64], dt.float32)
            nc.vector.tensor_tensor(out=cy2[:], in0=cy[:], in1=cy[:], op=OP.mult)
            nc.vector.tensor_tensor(out=q[:], in0=q[:], in1=cy2[:], op=OP.add)
            nc.vector.tensor_scalar_max(q[:], q[:], 1e-24)
            iv = sb.tile([P, 64], dt.int32)
            nc.vector.tensor_scalar(out=iv[:], in0=q[:].bitcast(dt.int32), scalar1=1,
                                    scalar2=None, op0=OP.logical_shift_right)
            nc.vector.tensor_tensor(out=iv[:], in0=magic[:], in1=iv[:], op=OP.subtract)
            y = iv.bitcast(dt.float32)
            u = sb.tile([P, 64], dt.float32)
            for _ in range(3):
                nc.vector.tensor_tensor(out=u[:], in0=y[:], in1=y[:], op=OP.mult)
                nc.vector.tensor_tensor(out=u[:], in0=u[:], in1=q[:], op=OP.mult)
                nc.vector.tensor_scalar(out=u[:], in0=u[:], scalar1=-0.5, scalar2=1.5,
                                        op0=OP.mult, op1=OP.add)
                nc.vector.tensor_tensor(out=y[:], in0=y[:], in1=u[:], op=OP.mult)
            nc.vector.tensor_tensor(out=cx[:], in0=cx[:], in1=y[:], op=OP.mult)
            nc.vector.tensor_tensor(out=cy[:], in0=cy[:], in1=y[:], op=OP.mult)
            nc.sync.dma_start(out=CX[:], in_=cx[:])
            nc.sync.dma_start(out=CY[:], in_=cy[:])
    nc.finalize()
    return nc


# ----------------------------------------------------------------------------
# orchestration
# ----------------------------------------------------------------------------

_REP16 = np.zeros((H, P), np.float32)
for _h in range(H):
    _REP16[_h, _h * 16 : (_h + 1) * 16] = 1.0


def kernel(**inputs):
    from concourse.bass_utils import run_bass_kernel_spmd

    bf16 = np.float16
    f8 = ml_dtypes.float8_e4m3fn

    x = np.ascontiguousarray(np.asarray(inputs["x"], np.float32))
    traces = []

    def note(r):
        if r.instructions_and_trace:
            traces.append(r.instructions_and_trace[1])
        return r

    prep = host_prep(inputs["src"], inputs["dst"])
    order, K = prep["order"], prep["K"]
    cores = list(range(NCORES))
    cols = [core_cols(c) for c in cores]

    # per-layer basis folds
    M = {}
    B = {}
    ADP = {}
    for l in (1, 2, 3):
        M[l], B[l], ADP[l] = fold_basis(np.asarray(inputs[f"as{l}"], np.float32),
                                        np.asarray(inputs[f"ad{l}"], np.float32))

    xT = np.zeros((INP, N), np.float32)
    xT[:IN] = x[order].T
    W1f = np.zeros((INP, HC), np.float32)
    W1f[:IN] = np.asarray(inputs["W1"], np.float32) @ M[1]
    W1q = W1f.astype(bf16)

    # ---- P1 ----
    p1 = build_p1()
    in_maps = [dict(xT=np.ascontiguousarray(xT[:, cols[c]]).astype(bf16), W1=W1q)
               for c in cores]
    r1 = note(run_bass_kernel_spmd(p1, in_maps, cores))
    times = [r1.exec_time_ns]

    def assemble(slabs):
        Tf = np.zeros((INP, HC), bf16)
        for c in cores:
            Tf[cols[c]] = slabs[c]
        Tf[PAD_IDX] = pad_row().astype(bf16)
        return Tf

    Tfull = assemble([r1.results[c]["Tout"] for c in cores])

    # ---- P2 (layers 2, 3) ----
    p2 = build_p23(K, with_next=True, with_head=False)
    xprev = [np.zeros((P, 1024), np.float32) for _ in cores]
    for l in (2, 3):
        adpr = np.broadcast_to(ADP[l - 1].astype(bf16), (P, HC)).copy()
        Wn = np.ascontiguousarray(np.asarray(inputs[f"W{l}"], np.float32) @ M[l])
        in_maps = []
        for c in cores:
            in_maps.append(dict(
                Tfull=Tfull, Town=np.ascontiguousarray(Tfull[cols[c]]),
                xprev=xprev[c], idxq=prep["idxq"][c],
                bprev=np.asarray(inputs[f"b{l-1}"], np.float32).reshape(P, 1),
                gam=np.asarray(inputs[f"g{l-1}"], np.float32).reshape(P, 1),
                bet=np.asarray(inputs[f"be{l-1}"], np.float32).reshape(P, 1),
                adpr=adpr, Bmat=B[l - 1], rep16q=_REP16, Wn=Wn,
            ))
        r2 = note(run_bass_kernel_spmd(p2, in_maps, cores))
        times.append(r2.exec_time_ns)
        Tfull = assemble([r2.results[c]["Tout"] for c in cores])
        for c in cores:
            xprev[c] = r2.results[c]["xnout"]

    # ---- P3 (layer-3 aggregation + MLP head) ----
    p3 = build_p23(K, with_next=False, with_head=True)
    adpr3 = np.broadcast_to(ADP[3].astype(bf16), (P, HC)).copy()
    in_maps = []
    for c in cores:
        in_maps.append(dict(
            Tfull=Tfull, Town=np.ascontiguousarray(Tfull[cols[c]]),
            xprev=xprev[c], idxq=prep["idxq"][c],
            bprev=np.asarray(inputs["b3"], np.float32).reshape(P, 1),
            gam=np.asarray(inputs["g3"], np.float32).reshape(P, 1),
            bet=np.asarray(inputs["be3"], np.float32).reshape(P, 1),
            adpr=adpr3, Bmat=B[3], rep16q=_REP16,
            aW1=np.ascontiguousarray(np.asarray(inputs["aW1"], np.float32)),
            ab1=np.asarray(inputs["ab1"], np.float32).reshape(P, 1),
            agm=np.asarray(inputs["ag"], np.float32).reshape(P, 1),
            abe=np.asarray(inputs["abe"], np.float32).reshape(P, 1),
            aW2=np.asarray(inputs["aW2"], np.float32).reshape(P, 1),
            ab2=np.asarray(inputs["ab2"], np.float32).reshape(1, 1),
            rW1=np.ascontiguousarray(np.asarray(inputs["rW1"], np.float32)),
            rb1=np.asarray(inputs["rb1"], np.float32).reshape(64, 1),
            rgm=np.asarray(inputs["rg"], np.float32).reshape(64, 1),
            rbe=np.asarray(inputs["rbe"], np.float32).reshape(64, 1),
            rW2=np.asarray(inputs["rW2"], np.float32).reshape(64, 1),
            rb2=np.asarray(inputs["rb2"], np.float32).reshape(1, 1),
        ))
    r3 = note(run_bass_kernel_spmd(p3, in_maps, cores))
    times.append(r3.exec_time_ns)

    out = np.zeros((N, 2), np.float32)
    for c in cores:
        nodes = order[cols[c]]
        out[nodes, 0] = r3.results[c]["COX"].reshape(1024)
        out[nodes, 1] = r3.results[c]["COY"].reshape(1024)
    kernel._last_times = times
    kernel._last_traces = traces
    return out
